# revision 25
# baseline (speedup 1.0000x reference)
"""Distributed Trainium2 Bass kernel for nn_Attention (LN + fused QKV + RoPE +
MHA-with-in-proj + out-proj), SPMD over 8 NeuronCores.

Sharding: both batches sequence-sharded across 8 cores; core c owns rows
[256c, 256c+256) of batch 0 AND batch 1 (512 tokens/core). Projections run on
the mixed 512-token block; attention runs per batch (256 queries x 2048 keys).
K-heads and V rows are exchanged via bf16 AllGathers on the (serial) CC
stream, ordered AG(kh pairs 0-3) -> AG(vh) -> AG(kh pairs 4-7) so the vh
gather lands while the first head-pairs' softmax runs.

Design notes:
 - bf16 everywhere on the matmul path, f32 PSUM. rel err ~1e-2 (budget 2e-2).
 - LayerNorm folded into the qkv projections algebraically:
     qkv = rstd*(W.T x + (-s1) (x) murstd + b1 (x) sd),  s1 = colsum(W)
   one K=2 correction matmul per chunk, one DVE mul epilogue. Projections
   consume raw x, so nothing waits on the LN reduction.
 - PE chain order: kproj, rope-k(in place), k-inproj, vproj, vh, qproj,
   rope-q, q-inproj, attention, out-proj per batch.
 - gpsimd queue carries ONLY collectives (it head-blocks on their waits);
   all broadcasts are 0-partition-stride DMAs, all small moves DMA/DVE.
 - Attention: scores [keys, q] per (b, head); the two heads of a pair run as
   concurrent PE row-group matmuls (K=64 at partitions 0:64 / 64:128). exp on
   ACT (the true critical engine, ~150us); mask applied multiplicatively
   (exp(mask) precomputed). AV appends a ones-column producing the softmax
   denominator at partition 64; reciprocal_approx_fast + DMA-broadcast.
 - The first 4 (b0, hp) units run "scores+exp only" with their attention
   weights staged in SBUF, so ACT keeps running while AG(vh) is in flight;
   their AV matmuls replay from the staging once vh arrives.
"""

import numpy as np

import concourse.bass as bass
import concourse.tile as tile
from concourse import bacc, mybir
from concourse.bass_utils import run_bass_kernel_spmd

B, S, D = 2, 2048, 1024
H, HD = 16, 64
NCORES = 8
TPB = 256  # tokens per core per batch
T = 2 * TPB  # tokens per core
EPS = 1e-5
THETA = 10000.0
P = 128
F32 = mybir.dt.float32
BF16 = mybir.dt.bfloat16
Copy = mybir.ActivationFunctionType.Copy
Ident = mybir.ActivationFunctionType.Identity
Exp = mybir.ActivationFunctionType.Exp
Sqrt = mybir.ActivationFunctionType.Sqrt
MUL = mybir.AluOpType.mult
ADD = mybir.AluOpType.add
SUB = mybir.AluOpType.subtract

TRACE = False  # test.py flips this for profiling runs

N_STAGED = 4  # (b=0, hp<4) units whose attn weights are staged pre-AG2

_cached = {}


def _bcast_ap(src, nparts):
    """0-partition-stride AP replicating src's single partition nparts ways."""
    return bass.AP(
        tensor=src.tensor, offset=src.offset, ap=[[0, nparts]] + src.ap[1:]
    )


def _build_module():
    nc = bacc.Bacc(None, target_bir_lowering=False)

    xT = nc.declare_dram_parameter("xT", [D, T], BF16, isOutput=False)
    maskT = nc.declare_dram_parameter("maskT", [S, T], BF16, isOutput=False)
    cosT = nc.declare_dram_parameter("cosT", [D // 2, T], BF16, isOutput=False)
    sinT = nc.declare_dram_parameter("sinT", [D // 2, T], BF16, isOutput=False)
    w1qkT = nc.declare_dram_parameter("w1qkT", [D, 2 * D], BF16, isOutput=False)
    w1vT = nc.declare_dram_parameter("w1vT", [D, D], BF16, isOutput=False)
    c1qk = nc.declare_dram_parameter("c1qk", [2, 2 * D], BF16, isOutput=False)
    c1v = nc.declare_dram_parameter("c1v", [2, D], BF16, isOutput=False)
    w2T = nc.declare_dram_parameter("w2T", [D, 2 * D], BF16, isOutput=False)
    b2 = nc.declare_dram_parameter("b2", [1, 2 * D], BF16, isOutput=False)
    wvT = nc.declare_dram_parameter("wvT", [D, D], BF16, isOutput=False)
    bvr = nc.declare_dram_parameter("bvr", [1, D], BF16, isOutput=False)
    owT = nc.declare_dram_parameter("owT", [D, D], BF16, isOutput=False)
    outb = nc.declare_dram_parameter("outb", [1, D], BF16, isOutput=False)
    outT = nc.declare_dram_parameter("outT", [D, T], F32, isOutput=True)

    RG = [list(range(NCORES))]

    with tile.TileContext(nc) as tc:
        with (
            tc.tile_pool(name="persist", bufs=1) as persist,
            tc.tile_pool(name="dram", bufs=1, space="DRAM") as dram,
        ):
            qhT = persist.tile([P, 8, T], BF16)  # head-pair-major q heads
            avT = persist.tile([P, 8, T], BF16)  # attention out, feature-major
            expm = persist.tile([P, 16, T], BF16)  # exp(mask), key-major
            c1qk_sb = persist.tile([2, 2 * D], BF16)
            c1v_sb = persist.tile([2, D], BF16)
            b2_sb = persist.tile([1, 2 * D], BF16)
            bvr_sb = persist.tile([1, D], BF16)
            outb_sb = persist.tile([1, D], BF16)
            ones_col = persist.tile([P, 1], BF16)
            ones_row = persist.tile([1, T], BF16)
            eps_sb = persist.tile([1, 1], F32)
            corr_rhs = persist.tile([2, T], BF16)  # row0=murstd row1=sd
            rstd_b = persist.tile([P, T], F32)

            ag1a_in = dram.tile([D // 2, T], BF16)
            ag1a_out = dram.tile([NCORES * D // 2, T], BF16, addr_space="Shared")
            ag1b_in = dram.tile([D // 2, T], BF16)
            ag1b_out = dram.tile([NCORES * D // 2, T], BF16, addr_space="Shared")
            ag2_in = dram.tile([T, D], BF16)
            ag2_out = dram.tile([NCORES * T, D], BF16, addr_space="Shared")
            bcd = dram.tile([2, T], BF16)  # DRAM bounce for corr_rhs rows
            rsd = dram.tile([1, T], F32)  # DRAM bounce for rstd broadcast

            with (
                tc.tile_pool(name="xpool", bufs=1) as xpool,
                tc.tile_pool(name="wpool", bufs=3) as wpool,
            ):
                # x first: it gates LN stats AND every projection matmul
                xfull = xpool.tile([P, 8, T], BF16)
                nc.sync.dma_start(xfull[:], xT.rearrange("(ko p) t -> p ko t", p=P))
                cos_sb = xpool.tile([P, 4, T], BF16)
                sin_sb = xpool.tile([P, 4, T], BF16)
                nc.sync.dma_start(cos_sb[:], cosT.rearrange("(c p) t -> p c t", p=P))
                nc.sync.dma_start(sin_sb[:], sinT.rearrange("(c p) t -> p c t", p=P))
                nc.sync.dma_start(c1qk_sb[:], c1qk[:])
                nc.sync.dma_start(c1v_sb[:], c1v[:])
                nc.sync.dma_start(b2_sb[:], b2[:])
                nc.sync.dma_start(bvr_sb[:], bvr[:])
                nc.sync.dma_start(outb_sb[:], outb[:])
                nc.vector.memset(ones_col[:], 1.0)
                nc.vector.memset(ones_row[:], 1.0)
                nc.vector.memset(eps_sb[:], EPS)

                # ---- LN statistics (sum / sum-of-squares via PE) ----
                with (
                    tc.tile_pool(name="sqp", bufs=2) as sqp,
                    tc.tile_pool(name="lnrows", bufs=1) as lnrows,
                    tc.tile_pool(name="psLN", bufs=2, space="PSUM") as psLN,
                ):
                    pt_s = psLN.tile([1, T], F32, tag="s")
                    pt_q = psLN.tile([1, T], F32, tag="q")
                    for ko in range(8):
                        sq = sqp.tile([P, T], BF16, tag="sq")
                        nc.vector.tensor_tensor(
                            sq[:], xfull[:, ko, :], xfull[:, ko, :], MUL
                        )
                        nc.tensor.matmul(
                            pt_s[0:1, :], ones_col[:], xfull[:, ko, :],
                            start=(ko == 0), stop=(ko == 7),
                        )
                        nc.tensor.matmul(
                            pt_q[0:1, :], ones_col[:], sq[:],
                            start=(ko == 0), stop=(ko == 7),
                        )
                    mu = lnrows.tile([1, T], F32)
                    msq = lnrows.tile([1, T], F32)
                    nc.vector.tensor_scalar_mul(mu[:], pt_s[0:1, :], 1.0 / D)
                    nc.vector.tensor_scalar_mul(msq[:], pt_q[0:1, :], 1.0 / D)
                    var = lnrows.tile([1, T], F32)
                    nc.vector.tensor_tensor(var[:], mu[:], mu[:], MUL)
                    nc.vector.tensor_tensor(var[:], msq[:], var[:], SUB)
                    sd = lnrows.tile([1, T], F32)
                    nc.scalar.activation(
                        out=sd[:], in_=var[:], func=Sqrt, bias=eps_sb[:]
                    )
                    rstd = lnrows.tile([1, T], F32)
                    nc.vector.reciprocal_approx_fast(out=rstd[:], in_=sd[:])
                    murstd = lnrows.tile([1, T], F32)
                    nc.vector.tensor_tensor(murstd[:], mu[:], rstd[:], MUL)
                    sdb = lnrows.tile([1, T], BF16)
                    nc.vector.tensor_copy(sdb[:], sd[:])
                    murb = lnrows.tile([1, T], BF16)
                    nc.vector.tensor_copy(murb[:], murstd[:])
                    # partition placement / broadcast via DRAM (0-stride
                    # partition APs are only legal on DRAM sources)
                    nc.sync.dma_start(bcd[0:1, :], murb[:])
                    nc.sync.dma_start(bcd[1:2, :], sdb[:])
                    nc.sync.dma_start(corr_rhs[0:2, :], bcd[0:2, :])
                    nc.sync.dma_start(rsd[0:1, :], rstd[:])
                    nc.sync.dma_start(rstd_b[:], _bcast_ap(rsd[0:1, :], P))

                w1view = w1qkT.rearrange("(ko p) j -> p ko j", p=P)
                w1vview = w1vT.rearrange("(ko p) j -> p ko j", p=P)
                w2view = w2T.rearrange("(ko p) j -> p ko j", p=P)
                owview = owT.rearrange("(ko p) j -> p ko j", p=P)

                with (
                    tc.tile_pool(name="qk", bufs=2) as qkp,
                    tc.tile_pool(name="rtmp", bufs=2) as rtmp,
                    tc.tile_pool(name="stage", bufs=2) as stagep,
                    tc.tile_pool(name="psA", bufs=3, space="PSUM") as psA,
                ):

                    def project_ln(dst, dst_ko, wview, jcol, corr_sb):
                        """dst[:,dst_ko,:] = rstd*(W.T x + corr.T [murstd; sd])."""
                        wt = wpool.tile([P, 8, P], BF16, tag="w")
                        nc.sync.dma_start(wt[:], wview[:, :, jcol : jcol + P])
                        pt = psA.tile([P, T], F32, tag="proj")
                        for ko in range(8):
                            nc.tensor.matmul(
                                pt[:], wt[:, ko, :], xfull[:, ko, :],
                                start=(ko == 0), stop=False,
                            )
                        nc.tensor.matmul(
                            pt[:], corr_sb[0:2, jcol : jcol + P], corr_rhs[0:2, :],
                            start=False, stop=True,
                        )
                        nc.vector.tensor_tensor(
                            dst[:, dst_ko, :], pt[:], rstd_b[:], MUL
                        )

                    def project_plain(dst, dst_ko, wview, jcol, bias_sb, bofs, rhs):
                        """dst[:,dst_ko,:] = W.T rhs + bias."""
                        wt = wpool.tile([P, 8, P], BF16, tag="w")
                        nc.sync.dma_start(wt[:], wview[:, :, jcol : jcol + P])
                        pt = psA.tile([P, T], F32, tag="proj")
                        for ko in range(8):
                            nc.tensor.matmul(
                                pt[:], wt[:, ko, :], rhs[:, ko, :],
                                start=(ko == 0), stop=False,
                            )
                        nc.tensor.matmul(
                            pt[:], bias_sb[0:1, bofs : bofs + P], ones_row[:],
                            start=False, stop=True,
                        )
                        nc.vector.tensor_copy(dst[:, dst_ko, :], pt[:])

                    def rope_inplace(src):
                        # src[c], src[4+c] <- rotated pair (in place)
                        for c in range(4):
                            x1 = src[:, c, :]
                            x2 = src[:, 4 + c, :]
                            ta = rtmp.tile([P, T], BF16, tag="ra")
                            tb = rtmp.tile([P, T], BF16, tag="rb")
                            tc2 = rtmp.tile([P, T], BF16, tag="ra")
                            td = rtmp.tile([P, T], BF16, tag="rb")
                            nc.vector.tensor_tensor(ta[:], x1, cos_sb[:, c, :], MUL)
                            nc.vector.tensor_tensor(tb[:], x2, sin_sb[:, c, :], MUL)
                            nc.vector.tensor_tensor(tc2[:], x2, cos_sb[:, c, :], MUL)
                            nc.vector.tensor_tensor(td[:], x1, sin_sb[:, c, :], MUL)
                            nc.vector.tensor_tensor(x1, ta[:], tb[:], SUB)
                            nc.vector.tensor_tensor(x2, tc2[:], td[:], ADD)

                    # ---- k chain ----
                    kT = qkp.tile([P, 8, T], BF16, tag="qk")
                    for jm in range(8):
                        project_ln(kT, jm, w1view, D + P * jm, c1qk_sb)
                    rope_inplace(kT)
                    khT = stagep.tile([P, 8, T], BF16, tag="stage")
                    for jm in range(8):
                        project_plain(khT, jm, w2view, D + P * jm, b2_sb, D + P * jm, kT)
                        if jm == 3:
                            nc.sync.dma_start(
                                ag1a_in.rearrange("(ko p) t -> p ko t", p=P),
                                khT[:, 0:4, :],
                            )
                            cc_a1a = nc.gpsimd.collective_compute(
                                "AllGather", mybir.AluOpType.bypass,
                                ins=[ag1a_in.opt()], outs=[ag1a_out.opt()],
                                replica_groups=RG,
                            )

                    # exp(mask): emitted here so its DMAs sit behind the k-chain
                    # weight loads; ACT does these while PE projects
                    with tc.tile_pool(name="mload", bufs=2) as mload:
                        mview = maskT.rearrange("(jc p) t -> p jc t", p=P)
                        for g in range(4):
                            mt = mload.tile([P, 4, T], BF16)
                            nc.sync.dma_start(mt[:], mview[:, 4 * g : 4 * g + 4, :])
                            nc.scalar.activation(
                                out=expm[:, 4 * g : 4 * g + 4, :], in_=mt[:],
                                func=Exp,
                            )

                    # ---- q chain ----
                    qT = qkp.tile([P, 8, T], BF16, tag="qk")
                    for jm in range(8):
                        project_ln(qT, jm, w1view, P * jm, c1qk_sb)
                    rope_inplace(qT)
                    for jm in range(8):
                        project_plain(qhT, jm, w2view, P * jm, b2_sb, P * jm, qT)

                    # ---- v chain (vh AG goes second on the CC stream) ----
                    vT = qkp.tile([P, 8, T], BF16, tag="qk")
                    for jm in range(8):
                        project_ln(vT, jm, w1vview, P * jm, c1v_sb)
                    with tc.tile_pool(name="wvp", bufs=2) as wvp:
                        vh_bf = stagep.tile([P, 4, D], BF16, tag="stage")
                        wvview = wvT.rearrange("(ko p) n -> p ko n", p=P)
                        for nh in range(4):
                            wv_rhs = wvp.tile([P, 8, 256], BF16)
                            nc.sync.dma_start(
                                wv_rhs[:], wvview[:, :, 256 * nh : 256 * nh + 256]
                            )
                            for tm in range(4):
                                pt = psA.tile([P, T], F32, tag="proj")
                                for ko in range(8):
                                    nc.tensor.matmul(
                                        pt[:, 0:256],
                                        vT[:, ko, P * tm : P * tm + P],
                                        wv_rhs[:, ko, :],
                                        start=(ko == 0), stop=False,
                                    )
                                nc.tensor.matmul(
                                    pt[:, 0:256],
                                    ones_row[0:1, 0:P],
                                    bvr_sb[0:1, 256 * nh : 256 * nh + 256],
                                    start=False, stop=True,
                                )
                                nc.vector.tensor_copy(
                                    vh_bf[:, tm, 256 * nh : 256 * nh + 256],
                                    pt[:, 0:256],
                                )
                        nc.sync.dma_start(
                            ag2_in.rearrange("(tm p) n -> p tm n", p=P),
                            vh_bf[:],
                        )
                        cc_ag2 = nc.gpsimd.collective_compute(
                            "AllGather", mybir.AluOpType.bypass,
                            ins=[ag2_in.opt()], outs=[ag2_out.opt()],
                            replica_groups=RG,
                        )

                    # kh pairs 4-7 gather last (not needed until ~exp midpoint);
                    # force it behind AG2 on the serial CC stream
                    nc.sync.dma_start(
                        ag1b_in.rearrange("(ko p) t -> p ko t", p=P),
                        khT[:, 4:8, :],
                    )
                    cc_a1b = nc.gpsimd.collective_compute(
                        "AllGather", mybir.AluOpType.bypass,
                        ins=[ag1b_in.opt()], outs=[ag1b_out.opt()],
                        replica_groups=RG,
                    )

                # ---- attention ----
                # ag1{a,b}_out rows: 512*r + 128*hp' + 64*sub + hd
                # ag2_out rows: 512*r + 256*b + tok ; cols 64*h + hd
                kviewA = ag1a_out.rearrange(
                    "(r hp sub hd) t -> hp (sub hd) r t", hp=4, sub=2, hd=HD
                )
                kviewB = ag1b_out.rearrange(
                    "(r hp sub hd) t -> hp (sub hd) r t", hp=4, sub=2, hd=HD
                )
                vview = ag2_out.rearrange(
                    "(r b2 half p) f -> b2 half p r f", b2=2, half=2, p=P
                )
                with (
                    tc.tile_pool(name="kload", bufs=2) as kload,
                    tc.tile_pool(name="vload", bufs=2) as vload,
                    tc.tile_pool(name="aep", bufs=4) as aep,
                    tc.tile_pool(name="atp", bufs=8 * N_STAGED + 2) as atp,
                    tc.tile_pool(name="nrm", bufs=2) as nrm,
                    tc.tile_pool(name="oc", bufs=2) as ocp,
                    tc.tile_pool(name="psS", bufs=2, space="PSUM") as psS,
                    tc.tile_pool(name="psV", bufs=2, space="PSUM") as psV,
                    tc.tile_pool(name="psD", bufs=1, space="PSUM") as psD,
                ):

                    def load_kp(b, hp):
                        kview = kviewA if hp < 4 else kviewB
                        kp = kload.tile([P, 8, TPB], BF16, tag="kp")
                        nc.sync.dma_start(
                            kp[:], kview[hp % 4][:, :, TPB * b : TPB * b + TPB]
                        )
                        return kp

                    def load_vh(b, hp):
                        vh_sb = vload.tile([P, 8, 2, 2, HD + 1], BF16, tag="vh")
                        for half in range(2):
                            for sub in range(2):
                                c0 = P * hp + HD * sub
                                nc.sync.dma_start(
                                    vh_sb[:, :, half, sub, 0:HD],
                                    vview[b][half][:, :, c0 : c0 + HD],
                                )
                        nc.vector.memset(vh_sb[:, :, :, :, HD : HD + 1], 1.0)
                        return vh_sb

                    def scores_pair(b, hp, kp):
                        """scores+exp+mask for both heads; returns 4x2 attnT."""
                        ats = []
                        for g in range(4):
                            s_pt0 = psS.tile([P, 4, TPB], F32, tag="s")
                            s_pt1 = psS.tile([P, 4, TPB], F32, tag="s")
                            s_pt = [s_pt0, s_pt1]
                            for u in range(4):
                                jc = 4 * g + u
                                r, half = jc // 2, jc % 2
                                for sub in range(2):
                                    h0 = HD * sub
                                    nc.tensor.matmul(
                                        s_pt[sub][:, u, :],
                                        kp[h0 : h0 + HD, r, P * half : P * half + P],
                                        qhT[h0 : h0 + HD, hp, TPB * b : TPB * b + TPB],
                                        start=True, stop=True,
                                    )
                            for sub in range(2):
                                attnE = aep.tile([P, 4, TPB], BF16, tag="ae")
                                nc.scalar.activation(
                                    out=attnE[:], in_=s_pt[sub][:], func=Exp
                                )
                                attnT = atp.tile([P, 4, TPB], BF16, tag="at")
                                nc.vector.tensor_tensor(
                                    attnT[:],
                                    attnE[:],
                                    expm[:, 4 * g : 4 * g + 4, TPB * b : TPB * b + TPB],
                                    MUL,
                                )
                                ats.append((g, sub, attnT))
                        return ats

                    def av_pair(b, hp, vh_sb, ats):
                        av_pt0 = psV.tile([P, TPB], F32, tag="av")
                        av_pt1 = psV.tile([P, TPB], F32, tag="av")
                        av_pt = [av_pt0, av_pt1]
                        for g, sub, attnT in ats:
                            for u in range(4):
                                jc = 4 * g + u
                                nc.tensor.matmul(
                                    av_pt[sub][0 : HD + 1, :],
                                    vh_sb[:, jc // 2, jc % 2, sub, 0 : HD + 1],
                                    attnT[:, u, :],
                                    start=(g == 0 and u == 0),
                                    stop=(g == 3 and u == 3),
                                )
                        norm_pair(b, hp, av_pt)

                    def norm_pair(b, hp, av_pt):
                        for sub in range(2):
                            avs = nrm.tile([P, TPB], F32, tag="avs")
                            nc.vector.tensor_copy(
                                avs[0 : HD + 1, :], av_pt[sub][0 : HD + 1, :]
                            )
                            drow = nrm.tile([1, TPB], F32, tag="dr")
                            nc.sync.dma_start(drow[:], avs[HD : HD + 1, :])
                            rrow = nrm.tile([1, TPB], F32, tag="rr")
                            nc.vector.reciprocal_approx_fast(
                                out=rrow[:], in_=drow[:]
                            )
                            rb = nrm.tile([HD, TPB], F32, tag="rbt")
                            nc.gpsimd.partition_broadcast(rb[:], rrow[:])
                            if sub == 0:
                                nc.vector.tensor_tensor(
                                    avT[0:HD, hp, TPB * b : TPB * b + TPB],
                                    avs[0:HD, :], rb[:], MUL,
                                )
                            else:
                                avn = nrm.tile([HD, TPB], BF16, tag="avn")
                                nc.vector.tensor_tensor(
                                    avn[:], avs[0:HD, :], rb[:], MUL
                                )
                                nc.sync.dma_start(
                                    avT[HD:P, hp, TPB * b : TPB * b + TPB],
                                    avn[:],
                                )

                    def outproj(b):
                        for om in range(8):
                            owt = wpool.tile([P, 8, P], BF16, tag="w")
                            nc.sync.dma_start(
                                owt[:], owview[:, :, P * om : P * om + P]
                            )
                            pt = psV.tile([P, TPB], F32, tag="av")
                            for ko in range(8):
                                nc.tensor.matmul(
                                    pt[:],
                                    owt[:, ko, :],
                                    avT[:, ko, TPB * b : TPB * b + TPB],
                                    start=(ko == 0), stop=False,
                                )
                            nc.tensor.matmul(
                                pt[:],
                                outb_sb[0:1, P * om : P * om + P],
                                ones_row[0:1, 0:TPB],
                                start=False, stop=True,
                            )
                            oc = ocp.tile([P, TPB], F32, tag="oc")
                            nc.vector.tensor_copy(oc[:], pt[:])
                            nc.sync.dma_start(
                                outT.rearrange("(ko p) t -> p ko t", p=P)[
                                    :, om, TPB * b : TPB * b + TPB
                                ],
                                oc[:],
                            )

                    # Phase A: stage scores/exp for (b0, hp<N_STAGED) while the
                    # vh AllGather is still in flight
                    staged = []
                    for hp in range(N_STAGED):
                        kp = load_kp(0, hp)
                        staged.append(scores_pair(0, hp, kp))
                    # Phase B: replay their AVs (unblocks when AG2 lands)
                    for hp in range(N_STAGED):
                        vh_sb = load_vh(0, hp)
                        av_pair(0, hp, vh_sb, staged[hp])
                    staged = None
                    # Phase C: remaining units, interleaved normally.
                    # (b1, hp<4) next (kh already gathered), then hp>=4.
                    order = (
                        [(1, hp) for hp in range(4)]
                        + [(0, hp) for hp in range(N_STAGED, 4)]
                        + [(0, hp) for hp in range(4, 8)]
                        + [(1, hp) for hp in range(4, 8)]
                    )
                    done_b0 = False
                    for b, hp in order:
                        kp = load_kp(b, hp)
                        vh_sb = load_vh(b, hp)
                        # interleaved scores/exp/AV with filler matmuls into an
                        # unused av_pt row: keeps the PE activity window gapless
                        # so the HAM clock-gate stays at full rate
                        av_pt0 = psV.tile([P, TPB], F32, tag="av")
                        av_pt1 = psV.tile([P, TPB], F32, tag="av")
                        av_pt = [av_pt0, av_pt1]
                        dum_pt = psD.tile([1, T], F32, tag="dum")
                        for g in range(4):
                            s_pt0 = psS.tile([P, 4, TPB], F32, tag="s")
                            s_pt1 = psS.tile([P, 4, TPB], F32, tag="s")
                            s_pt = [s_pt0, s_pt1]
                            for u in range(4):
                                jc = 4 * g + u
                                r, half = jc // 2, jc % 2
                                for sub in range(2):
                                    h0 = HD * sub
                                    nc.tensor.matmul(
                                        s_pt[sub][:, u, :],
                                        kp[h0 : h0 + HD, r, P * half : P * half + P],
                                        qhT[h0 : h0 + HD, hp, TPB * b : TPB * b + TPB],
                                        start=True, stop=True,
                                    )
                            for f in range(4):
                                nc.tensor.matmul(
                                    dum_pt[0:1, :],
                                    ones_col[:],
                                    expm[:, f, :],
                                    start=True, stop=True,
                                    skip_group_check=True,
                                )
                            for sub in range(2):
                                attnE = aep.tile([P, 4, TPB], BF16, tag="ae")
                                nc.scalar.activation(
                                    out=attnE[:], in_=s_pt[sub][:], func=Exp
                                )
                                attnT = atp.tile([P, 4, TPB], BF16, tag="at")
                                nc.vector.tensor_tensor(
                                    attnT[:],
                                    attnE[:],
                                    expm[:, 4 * g : 4 * g + 4, TPB * b : TPB * b + TPB],
                                    MUL,
                                )
                                for u in range(4):
                                    jc = 4 * g + u
                                    nc.tensor.matmul(
                                        av_pt[sub][0 : HD + 1, :],
                                        vh_sb[:, jc // 2, jc % 2, sub, 0 : HD + 1],
                                        attnT[:, u, :],
                                        start=(g == 0 and u == 0),
                                        stop=(g == 3 and u == 3),
                                    )
                        norm_pair(b, hp, av_pt)
                        if b == 0 and hp == 7:
                            outproj(0)
                            done_b0 = True
                    assert done_b0
                    outproj(1)

    nc.finalize()
    return nc


def _bf16(x):
    x = np.ascontiguousarray(np.asarray(x, np.float32))
    u = x.view(np.uint32)
    r = ((u >> 16) & 1).astype(np.uint32)
    return ((u + 0x7FFF + r) & 0xFFFF0000).view(np.float32)


def _host_prep(x, mask, ln_g, ln_b, w_qkv, b_qkv, in_w, in_b, out_w, out_b):
    f32 = np.float32
    import ml_dtypes

    def to_bf(a):
        return np.asarray(a, np.float32).astype(ml_dtypes.bfloat16)

    perm = np.concatenate([np.arange(0, D, 2), np.arange(1, D, 2)])
    W1 = (w_qkv * ln_g[None, :]).astype(f32)
    b1 = (b_qkv + w_qkv @ ln_b).astype(f32)
    W1q, W1k, W1v = W1[0:D], W1[D : 2 * D], W1[2 * D :]
    b1q, b1k, b1v = b1[0:D], b1[D : 2 * D], b1[2 * D :]
    w1qkT = _bf16(np.concatenate([W1q[perm], W1k[perm]], axis=0).T)  # (D,2D)
    c1qk = np.stack(
        [-w1qkT.sum(axis=0), np.concatenate([b1q[perm], b1k[perm]])]
    ).astype(f32)  # (2, 2D): row0=-s1 row1=b1
    w1vT = _bf16(W1v.T)
    c1v = np.stack([-w1vT.sum(axis=0), b1v]).astype(f32)

    wq, wk, wv = in_w[0:D], in_w[D : 2 * D], in_w[2 * D :]
    bq, bk, bv = in_b[0:D], in_b[D : 2 * D], in_b[2 * D :]
    SC = 1.0 / np.sqrt(HD)
    w2q = np.ascontiguousarray((wq * SC).T[perm])
    w2k = np.ascontiguousarray(wk.T[perm])
    w2T = np.concatenate([w2q, w2k], axis=1).astype(f32)
    b2r = np.concatenate([bq * SC, bk]).reshape(1, 2 * D).astype(f32)
    wvT2 = np.ascontiguousarray(wv.T).astype(f32)
    bvr = bv.reshape(1, D).astype(f32)
    owT = np.ascontiguousarray(out_w.T).astype(f32)
    outbr = out_b.reshape(1, D).astype(f32)

    inv_freq = 1.0 / (THETA ** (np.arange(0, D, 2, dtype=np.float64) / D))

    shared = dict(
        w1qkT=to_bf(w1qkT), w1vT=to_bf(w1vT), c1qk=to_bf(c1qk), c1v=to_bf(c1v),
        w2T=to_bf(w2T), b2=to_bf(b2r), wvT=to_bf(wvT2), bvr=to_bf(bvr),
        owT=to_bf(owT), outb=to_bf(outbr),
    )
    in_maps = []
    for c in range(NCORES):
        rows = slice(TPB * c, TPB * c + TPB)
        xc = np.ascontiguousarray(
            np.concatenate([x[0, rows], x[1, rows]], axis=0).T
        )
        mc = np.ascontiguousarray(
            np.concatenate([mask[0, rows].T, mask[1, rows].T], axis=1)
        )
        pos = np.arange(TPB * c, TPB * c + TPB, dtype=np.float64)
        ang = inv_freq[:, None] * pos[None, :]  # (512, 256)
        cosc = np.cos(ang)
        sinc = np.sin(ang)
        m = dict(shared)
        m["xT"] = to_bf(xc)
        m["maskT"] = to_bf(mc)
        m["cosT"] = to_bf(np.concatenate([cosc, cosc], axis=1))
        m["sinT"] = to_bf(np.concatenate([sinc, sinc], axis=1))
        in_maps.append(m)
    return in_maps


def kernel(**inputs):
    if "nc" not in _cached:
        _cached["nc"] = _build_module()
    nc = _cached["nc"]
    in_maps = _host_prep(**inputs)
    res = run_bass_kernel_spmd(nc, in_maps, list(range(NCORES)), trace=TRACE)
    _cached["last_result"] = res
    out = np.empty((B, S, D), dtype=np.float32)
    for c in range(NCORES):
        o = res.results[c]["outT"]  # (D, 512)
        rows = slice(TPB * c, TPB * c + TPB)
        out[0, rows] = o[:, 0:TPB].T
        out[1, rows] = o[:, TPB : 2 * TPB].T
    return out


# revision 31
# speedup vs baseline: 1.0654x; 1.0654x over previous
"""Distributed Trainium2 Bass kernel for nn_Attention (LN + fused QKV + RoPE +
MHA-with-in-proj + out-proj), SPMD over 8 NeuronCores.

Sharding: both batches sequence-sharded across 8 cores; core c owns rows
[256c, 256c+256) of batch 0 AND batch 1 (512 tokens/core). Projections run on
the mixed 512-token block; attention runs per batch (256 queries x 2048 keys).
K-heads and V rows are exchanged via bf16 AllGathers on the (serial) CC
stream, ordered AG(kh pairs 0-3) -> AG(vh) -> AG(kh pairs 4-7) so the vh
gather lands while the first head-pairs' softmax runs.

Design notes:
 - bf16 everywhere on the matmul path, f32 PSUM. rel err ~1e-2 (budget 2e-2).
 - LayerNorm folded into the qkv projections algebraically:
     qkv = rstd*(W.T x + (-s1) (x) murstd + b1 (x) sd),  s1 = colsum(W)
   one K=2 correction matmul per chunk, one DVE mul epilogue. Projections
   consume raw x, so nothing waits on the LN reduction.
 - PE chain order: kproj, rope-k(in place), k-inproj, vproj, vh, qproj,
   rope-q, q-inproj, attention, out-proj per batch.
 - gpsimd queue carries ONLY collectives (it head-blocks on their waits);
   all broadcasts are 0-partition-stride DMAs, all small moves DMA/DVE.
 - Attention: scores [keys, q] per (b, head); the two heads of a pair run as
   concurrent PE row-group matmuls (K=64 at partitions 0:64 / 64:128). exp on
   ACT (the true critical engine, ~150us); mask applied multiplicatively
   (exp(mask) precomputed). AV appends a ones-column producing the softmax
   denominator at partition 64; reciprocal_approx_fast + DMA-broadcast.
 - The first 4 (b0, hp) units run "scores+exp only" with their attention
   weights staged in SBUF, so ACT keeps running while AG(vh) is in flight;
   their AV matmuls replay from the staging once vh arrives.
"""

import numpy as np

import concourse.bass as bass
import concourse.tile as tile
from concourse import bacc, mybir
from concourse.bass_utils import run_bass_kernel_spmd

B, S, D = 2, 2048, 1024
H, HD = 16, 64
NCORES = 8
TPB = 256  # tokens per core per batch
T = 2 * TPB  # tokens per core
EPS = 1e-5
THETA = 10000.0
P = 128
F32 = mybir.dt.float32
BF16 = mybir.dt.bfloat16
Copy = mybir.ActivationFunctionType.Copy
Ident = mybir.ActivationFunctionType.Identity
Exp = mybir.ActivationFunctionType.Exp
Sqrt = mybir.ActivationFunctionType.Sqrt
MUL = mybir.AluOpType.mult
ADD = mybir.AluOpType.add
SUB = mybir.AluOpType.subtract

TRACE = False  # test.py flips this for profiling runs

N_STAGED = 4  # (b=0, hp<4) units whose attn weights are staged pre-AG2

_cached = {}


def _bcast_ap(src, nparts):
    """0-partition-stride AP replicating src's single partition nparts ways."""
    return bass.AP(
        tensor=src.tensor, offset=src.offset, ap=[[0, nparts]] + src.ap[1:]
    )


def _build_module():
    nc = bacc.Bacc(None, target_bir_lowering=False)

    xT = nc.declare_dram_parameter("xT", [D, T], BF16, isOutput=False)
    maskT = nc.declare_dram_parameter("maskT", [S, T], BF16, isOutput=False)
    cosT = nc.declare_dram_parameter("cosT", [D // 2, T], BF16, isOutput=False)
    sinT = nc.declare_dram_parameter("sinT", [D // 2, T], BF16, isOutput=False)
    w1qkT = nc.declare_dram_parameter("w1qkT", [D, 2 * D], BF16, isOutput=False)
    w1vT = nc.declare_dram_parameter("w1vT", [D, D], BF16, isOutput=False)
    c1qk = nc.declare_dram_parameter("c1qk", [2, 2 * D], BF16, isOutput=False)
    c1v = nc.declare_dram_parameter("c1v", [2, D], BF16, isOutput=False)
    w2T = nc.declare_dram_parameter("w2T", [D, 2 * D], BF16, isOutput=False)
    b2 = nc.declare_dram_parameter("b2", [1, 2 * D], BF16, isOutput=False)
    wvT = nc.declare_dram_parameter("wvT", [D, D], BF16, isOutput=False)
    bvr = nc.declare_dram_parameter("bvr", [1, D], BF16, isOutput=False)
    owT = nc.declare_dram_parameter("owT", [D, D], BF16, isOutput=False)
    outb = nc.declare_dram_parameter("outb", [1, D], BF16, isOutput=False)
    outT = nc.declare_dram_parameter("outT", [D, T], F32, isOutput=True)

    RG = [list(range(NCORES))]

    with tile.TileContext(nc) as tc:
        with (
            tc.tile_pool(name="persist", bufs=1) as persist,
            tc.tile_pool(name="dram", bufs=1, space="DRAM") as dram,
        ):
            qhT = persist.tile([P, 8, T], BF16)  # head-pair-major q heads
            qTp = persist.tile([P, 8, T], BF16)  # roped q, read by late q-inproj
            avT = persist.tile([P, 8, T], BF16)  # attention out, feature-major
            expm = persist.tile([P, 16, T], BF16)  # exp(mask), key-major
            c1qk_sb = persist.tile([2, 2 * D], BF16)
            c1v_sb = persist.tile([2, D], BF16)
            b2_sb = persist.tile([1, 2 * D], BF16)
            bvr_sb = persist.tile([1, D], BF16)
            outb_sb = persist.tile([1, D], BF16)
            ones_col = persist.tile([P, 1], BF16)
            ones_row = persist.tile([1, T], BF16)
            eps_sb = persist.tile([1, 1], F32)
            corr_rhs = persist.tile([2, T], BF16)  # row0=murstd row1=sd
            rstd_b = persist.tile([P, T], F32)

            ag1a_in = dram.tile([D // 2, T], BF16)
            ag1a_out = dram.tile([NCORES * D // 2, T], BF16, addr_space="Shared")
            ag1b_in = dram.tile([D // 2, T], BF16)
            ag1b_out = dram.tile([NCORES * D // 2, T], BF16, addr_space="Shared")
            ag2_in = dram.tile([T, D], BF16)
            ag2_out = dram.tile([NCORES * T, D], BF16, addr_space="Shared")
            bcd = dram.tile([2, T], BF16)  # DRAM bounce for corr_rhs rows
            rsd = dram.tile([1, T], F32)  # DRAM bounce for rstd broadcast

            with (
                tc.tile_pool(name="xpool", bufs=1) as xpool,
                tc.tile_pool(name="wpool", bufs=3) as wpool,
            ):
                # x first: it gates LN stats AND every projection matmul
                xfull = xpool.tile([P, 8, T], BF16)
                nc.sync.dma_start(xfull[:], xT.rearrange("(ko p) t -> p ko t", p=P))
                cos_sb = xpool.tile([P, 4, T], BF16)
                sin_sb = xpool.tile([P, 4, T], BF16)
                nc.sync.dma_start(cos_sb[:], cosT.rearrange("(c p) t -> p c t", p=P))
                nc.sync.dma_start(sin_sb[:], sinT.rearrange("(c p) t -> p c t", p=P))
                nc.sync.dma_start(c1qk_sb[:], c1qk[:])
                nc.sync.dma_start(c1v_sb[:], c1v[:])
                nc.sync.dma_start(b2_sb[:], b2[:])
                nc.sync.dma_start(bvr_sb[:], bvr[:])
                nc.sync.dma_start(outb_sb[:], outb[:])
                nc.vector.memset(ones_col[:], 1.0)
                nc.vector.memset(ones_row[:], 1.0)
                nc.vector.memset(eps_sb[:], EPS)

                # ---- LN statistics (sum / sum-of-squares via PE) ----
                with (
                    tc.tile_pool(name="sqp", bufs=2) as sqp,
                    tc.tile_pool(name="lnrows", bufs=1) as lnrows,
                    tc.tile_pool(name="psLN", bufs=2, space="PSUM") as psLN,
                ):
                    pt_s = psLN.tile([1, T], F32, tag="s")
                    pt_q = psLN.tile([1, T], F32, tag="q")
                    for ko in range(8):
                        sq = sqp.tile([P, T], BF16, tag="sq")
                        nc.vector.tensor_tensor(
                            sq[:], xfull[:, ko, :], xfull[:, ko, :], MUL
                        )
                        nc.tensor.matmul(
                            pt_s[0:1, :], ones_col[:], xfull[:, ko, :],
                            start=(ko == 0), stop=(ko == 7),
                        )
                        nc.tensor.matmul(
                            pt_q[0:1, :], ones_col[:], sq[:],
                            start=(ko == 0), stop=(ko == 7),
                        )
                    mu = lnrows.tile([1, T], F32)
                    msq = lnrows.tile([1, T], F32)
                    nc.vector.tensor_scalar_mul(mu[:], pt_s[0:1, :], 1.0 / D)
                    nc.vector.tensor_scalar_mul(msq[:], pt_q[0:1, :], 1.0 / D)
                    var = lnrows.tile([1, T], F32)
                    nc.vector.tensor_tensor(var[:], mu[:], mu[:], MUL)
                    nc.vector.tensor_tensor(var[:], msq[:], var[:], SUB)
                    sd = lnrows.tile([1, T], F32)
                    nc.scalar.activation(
                        out=sd[:], in_=var[:], func=Sqrt, bias=eps_sb[:]
                    )
                    rstd = lnrows.tile([1, T], F32)
                    nc.vector.reciprocal_approx_fast(out=rstd[:], in_=sd[:])
                    murstd = lnrows.tile([1, T], F32)
                    nc.vector.tensor_tensor(murstd[:], mu[:], rstd[:], MUL)
                    sdb = lnrows.tile([1, T], BF16)
                    nc.vector.tensor_copy(sdb[:], sd[:])
                    murb = lnrows.tile([1, T], BF16)
                    nc.vector.tensor_copy(murb[:], murstd[:])
                    # partition placement / broadcast via DRAM (0-stride
                    # partition APs are only legal on DRAM sources)
                    nc.sync.dma_start(bcd[0:1, :], murb[:])
                    nc.sync.dma_start(bcd[1:2, :], sdb[:])
                    nc.sync.dma_start(corr_rhs[0:2, :], bcd[0:2, :])
                    nc.sync.dma_start(rsd[0:1, :], rstd[:])
                    nc.sync.dma_start(rstd_b[:], _bcast_ap(rsd[0:1, :], P))

                w1view = w1qkT.rearrange("(ko p) j -> p ko j", p=P)
                w1vview = w1vT.rearrange("(ko p) j -> p ko j", p=P)
                w2view = w2T.rearrange("(ko p) j -> p ko j", p=P)
                owview = owT.rearrange("(ko p) j -> p ko j", p=P)

                with (
                    tc.tile_pool(name="qk", bufs=2) as qkp,
                    tc.tile_pool(name="rtmp", bufs=2) as rtmp,
                    tc.tile_pool(name="stage", bufs=2) as stagep,
                    tc.tile_pool(name="psA", bufs=3, space="PSUM") as psA,
                ):

                    def project_ln(dst, dst_ko, wview, jcol, corr_sb):
                        """dst[:,dst_ko,:] = rstd*(W.T x + corr.T [murstd; sd])."""
                        wt = wpool.tile([P, 8, P], BF16, tag="w")
                        nc.sync.dma_start(wt[:], wview[:, :, jcol : jcol + P])
                        pt = psA.tile([P, T], F32, tag="proj")
                        for ko in range(8):
                            nc.tensor.matmul(
                                pt[:], wt[:, ko, :], xfull[:, ko, :],
                                start=(ko == 0), stop=False,
                            )
                        nc.tensor.matmul(
                            pt[:], corr_sb[0:2, jcol : jcol + P], corr_rhs[0:2, :],
                            start=False, stop=True,
                        )
                        nc.vector.tensor_tensor(
                            dst[:, dst_ko, :], pt[:], rstd_b[:], MUL
                        )

                    def project_plain(dst, dst_ko, wview, jcol, bias_sb, bofs, rhs):
                        """dst[:,dst_ko,:] = W.T rhs + bias."""
                        wt = wpool.tile([P, 8, P], BF16, tag="w")
                        nc.sync.dma_start(wt[:], wview[:, :, jcol : jcol + P])
                        pt = psA.tile([P, T], F32, tag="proj")
                        for ko in range(8):
                            nc.tensor.matmul(
                                pt[:], wt[:, ko, :], rhs[:, ko, :],
                                start=(ko == 0), stop=False,
                            )
                        nc.tensor.matmul(
                            pt[:], bias_sb[0:1, bofs : bofs + P], ones_row[:],
                            start=False, stop=True,
                        )
                        nc.vector.tensor_copy(dst[:, dst_ko, :], pt[:])

                    def rope_inplace(src):
                        # src[c], src[4+c] <- rotated pair (in place)
                        for c in range(4):
                            x1 = src[:, c, :]
                            x2 = src[:, 4 + c, :]
                            ta = rtmp.tile([P, T], BF16, tag="ra")
                            tb = rtmp.tile([P, T], BF16, tag="rb")
                            tc2 = rtmp.tile([P, T], BF16, tag="ra")
                            td = rtmp.tile([P, T], BF16, tag="rb")
                            nc.vector.tensor_tensor(ta[:], x1, cos_sb[:, c, :], MUL)
                            nc.vector.tensor_tensor(tb[:], x2, sin_sb[:, c, :], MUL)
                            nc.vector.tensor_tensor(tc2[:], x2, cos_sb[:, c, :], MUL)
                            nc.vector.tensor_tensor(td[:], x1, sin_sb[:, c, :], MUL)
                            nc.vector.tensor_tensor(x1, ta[:], tb[:], SUB)
                            nc.vector.tensor_tensor(x2, tc2[:], td[:], ADD)

                    # ---- k chain ----
                    kT = qkp.tile([P, 8, T], BF16, tag="qk")
                    for jm in range(8):
                        project_ln(kT, jm, w1view, D + P * jm, c1qk_sb)
                    rope_inplace(kT)
                    khT = stagep.tile([P, 8, T], BF16, tag="stage")
                    for jm in range(8):
                        project_plain(khT, jm, w2view, D + P * jm, b2_sb, D + P * jm, kT)
                        if jm == 3:
                            nc.sync.dma_start(
                                ag1a_in.rearrange("(ko p) t -> p ko t", p=P),
                                khT[:, 0:4, :],
                            )
                            cc_a1a = nc.gpsimd.collective_compute(
                                "AllGather", mybir.AluOpType.bypass,
                                ins=[ag1a_in.opt()], outs=[ag1a_out.opt()],
                                replica_groups=RG,
                            )

                    # exp(mask): emitted here so its DMAs sit behind the k-chain
                    # weight loads; ACT does these while PE projects
                    with tc.tile_pool(name="mload", bufs=2) as mload:
                        mview = maskT.rearrange("(jc p) t -> p jc t", p=P)
                        for g in range(4):
                            mt = mload.tile([P, 4, T], BF16)
                            nc.sync.dma_start(mt[:], mview[:, 4 * g : 4 * g + 4, :])
                            nc.scalar.activation(
                                out=expm[:, 4 * g : 4 * g + 4, :], in_=mt[:],
                                func=Exp,
                            )

                    # ---- v chain (vh AG goes second on the CC stream) ----
                    vT = qkp.tile([P, 8, T], BF16, tag="qk")
                    for jm in range(8):
                        project_ln(vT, jm, w1vview, P * jm, c1v_sb)
                    with tc.tile_pool(name="wvp", bufs=2) as wvp:
                        vh_bf = stagep.tile([P, 4, D], BF16, tag="stage")
                        wvview = wvT.rearrange("(ko p) n -> p ko n", p=P)
                        for nh in range(4):
                            wv_rhs = wvp.tile([P, 8, 256], BF16)
                            nc.sync.dma_start(
                                wv_rhs[:], wvview[:, :, 256 * nh : 256 * nh + 256]
                            )
                            for tm in range(4):
                                pt = psA.tile([P, T], F32, tag="proj")
                                for ko in range(8):
                                    nc.tensor.matmul(
                                        pt[:, 0:256],
                                        vT[:, ko, P * tm : P * tm + P],
                                        wv_rhs[:, ko, :],
                                        start=(ko == 0), stop=False,
                                    )
                                nc.tensor.matmul(
                                    pt[:, 0:256],
                                    ones_row[0:1, 0:P],
                                    bvr_sb[0:1, 256 * nh : 256 * nh + 256],
                                    start=False, stop=True,
                                )
                                nc.vector.tensor_copy(
                                    vh_bf[:, tm, 256 * nh : 256 * nh + 256],
                                    pt[:, 0:256],
                                )
                        nc.sync.dma_start(
                            ag2_in.rearrange("(tm p) n -> p tm n", p=P),
                            vh_bf[:],
                        )
                        cc_ag2 = nc.gpsimd.collective_compute(
                            "AllGather", mybir.AluOpType.bypass,
                            ins=[ag2_in.opt()], outs=[ag2_out.opt()],
                            replica_groups=RG,
                        )

                    # ---- q chain ----
                    for jm in range(8):
                        project_ln(qTp, jm, w1view, P * jm, c1qk_sb)
                    rope_inplace(qTp)
                    # q in-projection happens inside the attention section,
                    # interleaved with the first score blocks

                    # kh pairs 4-7 gather last (not needed until ~exp midpoint);
                    # force it behind AG2 on the serial CC stream
                    nc.sync.dma_start(
                        ag1b_in.rearrange("(ko p) t -> p ko t", p=P),
                        khT[:, 4:8, :],
                    )
                    cc_a1b = nc.gpsimd.collective_compute(
                        "AllGather", mybir.AluOpType.bypass,
                        ins=[ag1b_in.opt()], outs=[ag1b_out.opt()],
                        replica_groups=RG,
                    )

                # ---- attention ----
                # ag1{a,b}_out rows: 512*r + 128*hp' + 64*sub + hd
                # ag2_out rows: 512*r + 256*b + tok ; cols 64*h + hd
                kviewA = ag1a_out.rearrange(
                    "(r hp sub hd) t -> hp (sub hd) r t", hp=4, sub=2, hd=HD
                )
                kviewB = ag1b_out.rearrange(
                    "(r hp sub hd) t -> hp (sub hd) r t", hp=4, sub=2, hd=HD
                )
                vview = ag2_out.rearrange(
                    "(r b2 half p) f -> b2 half p r f", b2=2, half=2, p=P
                )
                with (
                    tc.tile_pool(name="kload", bufs=2) as kload,
                    tc.tile_pool(name="vload", bufs=2) as vload,
                    tc.tile_pool(name="aep", bufs=4) as aep,
                    tc.tile_pool(name="atp", bufs=8 * N_STAGED + 2) as atp,
                    tc.tile_pool(name="nrm", bufs=2) as nrm,
                    tc.tile_pool(name="oc", bufs=2) as ocp,
                    tc.tile_pool(name="psS", bufs=2, space="PSUM") as psS,
                    tc.tile_pool(name="psV", bufs=2, space="PSUM") as psV,
                    tc.tile_pool(name="psD", bufs=1, space="PSUM") as psD,
                ):

                    def load_kp(b, hp):
                        kview = kviewA if hp < 4 else kviewB
                        kp = kload.tile([P, 8, TPB], BF16, tag="kp")
                        nc.sync.dma_start(
                            kp[:], kview[hp % 4][:, :, TPB * b : TPB * b + TPB]
                        )
                        return kp

                    def load_vh(b, hp):
                        vh_sb = vload.tile([P, 8, 2, 2, HD + 1], BF16, tag="vh")
                        for half in range(2):
                            for sub in range(2):
                                c0 = P * hp + HD * sub
                                nc.sync.dma_start(
                                    vh_sb[:, :, half, sub, 0:HD],
                                    vview[b][half][:, :, c0 : c0 + HD],
                                )
                        nc.vector.memset(vh_sb[:, :, :, :, HD : HD + 1], 1.0)
                        return vh_sb

                    def qin_chunk(jm):
                        """late q in-projection chunk (overlaps attention)."""
                        wt = wpool.tile([P, 8, P], BF16, tag="w")
                        nc.sync.dma_start(wt[:], w2view[:, :, P * jm : P * jm + P])
                        pt = psV.tile([P, T], F32, tag="av")
                        for ko in range(8):
                            nc.tensor.matmul(
                                pt[:], wt[:, ko, :], qTp[:, ko, :],
                                start=(ko == 0), stop=False,
                            )
                        nc.tensor.matmul(
                            pt[:], b2_sb[0:1, P * jm : P * jm + P], ones_row[:],
                            start=False, stop=True,
                        )
                        nc.vector.tensor_copy(qhT[:, jm, :], pt[:])

                    def scores_pair(b, hp, kp):
                        """scores+exp+mask for both heads; returns 4x2 attnT."""
                        ats = []
                        dum_pt = psD.tile([1, T], F32, tag="dum")
                        for g in range(4):
                            s_pt0 = psS.tile([P, 4, TPB], F32, tag="s")
                            s_pt1 = psS.tile([P, 4, TPB], F32, tag="s")
                            s_pt = [s_pt0, s_pt1]
                            for u in range(4):
                                jc = 4 * g + u
                                r, half = jc // 2, jc % 2
                                for sub in range(2):
                                    h0 = HD * sub
                                    nc.tensor.matmul(
                                        s_pt[sub][:, u, :],
                                        kp[h0 : h0 + HD, r, P * half : P * half + P],
                                        qhT[h0 : h0 + HD, hp, TPB * b : TPB * b + TPB],
                                        start=True, stop=True,
                                    )
                            for f in range(4):
                                nc.tensor.matmul(
                                    dum_pt[0:1, :],
                                    ones_col[:],
                                    expm[:, f, :],
                                    start=True, stop=True,
                                    skip_group_check=True,
                                )
                            for sub in range(2):
                                attnE = aep.tile([P, 4, TPB], BF16, tag="ae")
                                nc.scalar.activation(
                                    out=attnE[:], in_=s_pt[sub][:], func=Exp
                                )
                                attnT = atp.tile([P, 4, TPB], BF16, tag="at")
                                nc.vector.tensor_tensor(
                                    attnT[:],
                                    attnE[:],
                                    expm[:, 4 * g : 4 * g + 4, TPB * b : TPB * b + TPB],
                                    MUL,
                                )
                                ats.append((g, sub, attnT))
                        return ats

                    def av_pair(b, hp, vh_sb, ats):
                        av_pt0 = psV.tile([P, TPB], F32, tag="av")
                        av_pt1 = psV.tile([P, TPB], F32, tag="av")
                        av_pt = [av_pt0, av_pt1]
                        for g, sub, attnT in ats:
                            for u in range(4):
                                jc = 4 * g + u
                                nc.tensor.matmul(
                                    av_pt[sub][0 : HD + 1, :],
                                    vh_sb[:, jc // 2, jc % 2, sub, 0 : HD + 1],
                                    attnT[:, u, :],
                                    start=(g == 0 and u == 0),
                                    stop=(g == 3 and u == 3),
                                )
                        norm_pair(b, hp, av_pt)

                    def norm_pair(b, hp, av_pt):
                        for sub in range(2):
                            avs = nrm.tile([P, TPB], F32, tag="avs")
                            nc.vector.tensor_copy(
                                avs[0 : HD + 1, :], av_pt[sub][0 : HD + 1, :]
                            )
                            drow = nrm.tile([1, TPB], F32, tag="dr")
                            nc.sync.dma_start(drow[:], avs[HD : HD + 1, :])
                            rrow = nrm.tile([1, TPB], F32, tag="rr")
                            nc.vector.reciprocal_approx_fast(
                                out=rrow[:], in_=drow[:]
                            )
                            rb = nrm.tile([HD, TPB], F32, tag="rbt")
                            nc.gpsimd.partition_broadcast(rb[:], rrow[:])
                            if sub == 0:
                                nc.vector.tensor_tensor(
                                    avT[0:HD, hp, TPB * b : TPB * b + TPB],
                                    avs[0:HD, :], rb[:], MUL,
                                )
                            else:
                                avn = nrm.tile([HD, TPB], BF16, tag="avn")
                                nc.vector.tensor_tensor(
                                    avn[:], avs[0:HD, :], rb[:], MUL
                                )
                                nc.sync.dma_start(
                                    avT[HD:P, hp, TPB * b : TPB * b + TPB],
                                    avn[:],
                                )

                    def outproj(b, oms):
                        for om in oms:
                            owt = wpool.tile([P, 8, P], BF16, tag="w")
                            nc.sync.dma_start(
                                owt[:], owview[:, :, P * om : P * om + P]
                            )
                            pt = psV.tile([P, TPB], F32, tag="av")
                            for ko in range(8):
                                nc.tensor.matmul(
                                    pt[:],
                                    owt[:, ko, :],
                                    avT[:, ko, TPB * b : TPB * b + TPB],
                                    start=(ko == 0), stop=False,
                                )
                            nc.tensor.matmul(
                                pt[:],
                                outb_sb[0:1, P * om : P * om + P],
                                ones_row[0:1, 0:TPB],
                                start=False, stop=True,
                            )
                            oc = ocp.tile([P, TPB], F32, tag="oc")
                            nc.vector.tensor_copy(oc[:], pt[:])
                            nc.sync.dma_start(
                                outT.rearrange("(ko p) t -> p ko t", p=P)[
                                    :, om, TPB * b : TPB * b + TPB
                                ],
                                oc[:],
                            )

                    # Phase A: stage scores/exp for (b0, hp<N_STAGED) while the
                    # vh AllGather is still in flight; q in-projection chunks
                    # are produced just-in-time, interleaved with the scores
                    staged = []
                    for hp in range(N_STAGED):
                        qin_chunk(hp)
                        kp = load_kp(0, hp)
                        staged.append(scores_pair(0, hp, kp))
                    for jm in range(N_STAGED, 8):
                        qin_chunk(jm)
                    # Phase B: replay their AVs (unblocks when AG2 lands)
                    for hp in range(N_STAGED):
                        vh_sb = load_vh(0, hp)
                        av_pair(0, hp, vh_sb, staged[hp])
                    staged = None
                    # Phase C: remaining units, interleaved normally.
                    # (b1, hp<4) next (kh already gathered), then hp>=4.
                    order = (
                        [(1, hp) for hp in range(4)]
                        + [(0, hp) for hp in range(N_STAGED, 4)]
                        + [(0, hp) for hp in range(4, 8)]
                        + [(1, hp) for hp in range(4, 8)]
                    )
                    done_b0 = False
                    for b, hp in order:
                        kp = load_kp(b, hp)
                        vh_sb = load_vh(b, hp)
                        # interleaved scores/exp/AV with filler matmuls into an
                        # unused av_pt row: keeps the PE activity window gapless
                        # so the HAM clock-gate stays at full rate
                        av_pt0 = psV.tile([P, TPB], F32, tag="av")
                        av_pt1 = psV.tile([P, TPB], F32, tag="av")
                        av_pt = [av_pt0, av_pt1]
                        dum_pt = psD.tile([1, T], F32, tag="dum")
                        for g in range(4):
                            s_pt0 = psS.tile([P, 4, TPB], F32, tag="s")
                            s_pt1 = psS.tile([P, 4, TPB], F32, tag="s")
                            s_pt = [s_pt0, s_pt1]
                            for u in range(4):
                                jc = 4 * g + u
                                r, half = jc // 2, jc % 2
                                for sub in range(2):
                                    h0 = HD * sub
                                    nc.tensor.matmul(
                                        s_pt[sub][:, u, :],
                                        kp[h0 : h0 + HD, r, P * half : P * half + P],
                                        qhT[h0 : h0 + HD, hp, TPB * b : TPB * b + TPB],
                                        start=True, stop=True,
                                    )
                            for f in range(4):
                                nc.tensor.matmul(
                                    dum_pt[0:1, :],
                                    ones_col[:],
                                    expm[:, f, :],
                                    start=True, stop=True,
                                    skip_group_check=True,
                                )
                            for sub in range(2):
                                attnE = aep.tile([P, 4, TPB], BF16, tag="ae")
                                nc.scalar.activation(
                                    out=attnE[:], in_=s_pt[sub][:], func=Exp
                                )
                                attnT = atp.tile([P, 4, TPB], BF16, tag="at")
                                nc.vector.tensor_tensor(
                                    attnT[:],
                                    attnE[:],
                                    expm[:, 4 * g : 4 * g + 4, TPB * b : TPB * b + TPB],
                                    MUL,
                                )
                                for u in range(4):
                                    jc = 4 * g + u
                                    nc.tensor.matmul(
                                        av_pt[sub][0 : HD + 1, :],
                                        vh_sb[:, jc // 2, jc % 2, sub, 0 : HD + 1],
                                        attnT[:, u, :],
                                        start=(g == 0 and u == 0),
                                        stop=(g == 3 and u == 3),
                                    )
                        norm_pair(b, hp, av_pt)
                        if b == 0 and hp == 7:
                            done_b0 = True
                        # spread batch-0's out-projection over the last four
                        # units so it never starves ACT of score work
                        if b == 1 and hp >= 4:
                            assert done_b0
                            outproj(0, [2 * (hp - 4), 2 * (hp - 4) + 1])
                    outproj(1, range(8))

    nc.finalize()
    return nc


def _bf16(x):
    x = np.ascontiguousarray(np.asarray(x, np.float32))
    u = x.view(np.uint32)
    r = ((u >> 16) & 1).astype(np.uint32)
    return ((u + 0x7FFF + r) & 0xFFFF0000).view(np.float32)


def _host_prep(x, mask, ln_g, ln_b, w_qkv, b_qkv, in_w, in_b, out_w, out_b):
    f32 = np.float32
    import ml_dtypes

    def to_bf(a):
        return np.asarray(a, np.float32).astype(ml_dtypes.bfloat16)

    perm = np.concatenate([np.arange(0, D, 2), np.arange(1, D, 2)])
    W1 = (w_qkv * ln_g[None, :]).astype(f32)
    b1 = (b_qkv + w_qkv @ ln_b).astype(f32)
    W1q, W1k, W1v = W1[0:D], W1[D : 2 * D], W1[2 * D :]
    b1q, b1k, b1v = b1[0:D], b1[D : 2 * D], b1[2 * D :]
    w1qkT = _bf16(np.concatenate([W1q[perm], W1k[perm]], axis=0).T)  # (D,2D)
    c1qk = np.stack(
        [-w1qkT.sum(axis=0), np.concatenate([b1q[perm], b1k[perm]])]
    ).astype(f32)  # (2, 2D): row0=-s1 row1=b1
    w1vT = _bf16(W1v.T)
    c1v = np.stack([-w1vT.sum(axis=0), b1v]).astype(f32)

    wq, wk, wv = in_w[0:D], in_w[D : 2 * D], in_w[2 * D :]
    bq, bk, bv = in_b[0:D], in_b[D : 2 * D], in_b[2 * D :]
    SC = 1.0 / np.sqrt(HD)
    w2q = np.ascontiguousarray((wq * SC).T[perm])
    w2k = np.ascontiguousarray(wk.T[perm])
    w2T = np.concatenate([w2q, w2k], axis=1).astype(f32)
    b2r = np.concatenate([bq * SC, bk]).reshape(1, 2 * D).astype(f32)
    wvT2 = np.ascontiguousarray(wv.T).astype(f32)
    bvr = bv.reshape(1, D).astype(f32)
    owT = np.ascontiguousarray(out_w.T).astype(f32)
    outbr = out_b.reshape(1, D).astype(f32)

    inv_freq = 1.0 / (THETA ** (np.arange(0, D, 2, dtype=np.float64) / D))

    shared = dict(
        w1qkT=to_bf(w1qkT), w1vT=to_bf(w1vT), c1qk=to_bf(c1qk), c1v=to_bf(c1v),
        w2T=to_bf(w2T), b2=to_bf(b2r), wvT=to_bf(wvT2), bvr=to_bf(bvr),
        owT=to_bf(owT), outb=to_bf(outbr),
    )
    in_maps = []
    for c in range(NCORES):
        rows = slice(TPB * c, TPB * c + TPB)
        xc = np.ascontiguousarray(
            np.concatenate([x[0, rows], x[1, rows]], axis=0).T
        )
        mc = np.ascontiguousarray(
            np.concatenate([mask[0, rows].T, mask[1, rows].T], axis=1)
        )
        pos = np.arange(TPB * c, TPB * c + TPB, dtype=np.float64)
        ang = inv_freq[:, None] * pos[None, :]  # (512, 256)
        cosc = np.cos(ang)
        sinc = np.sin(ang)
        m = dict(shared)
        m["xT"] = to_bf(xc)
        m["maskT"] = to_bf(mc)
        m["cosT"] = to_bf(np.concatenate([cosc, cosc], axis=1))
        m["sinT"] = to_bf(np.concatenate([sinc, sinc], axis=1))
        in_maps.append(m)
    return in_maps


def kernel(**inputs):
    if "nc" not in _cached:
        _cached["nc"] = _build_module()
    nc = _cached["nc"]
    in_maps = _host_prep(**inputs)
    res = run_bass_kernel_spmd(nc, in_maps, list(range(NCORES)), trace=TRACE)
    _cached["last_result"] = res
    out = np.empty((B, S, D), dtype=np.float32)
    for c in range(NCORES):
        o = res.results[c]["outT"]  # (D, 512)
        rows = slice(TPB * c, TPB * c + TPB)
        out[0, rows] = o[:, 0:TPB].T
        out[1, rows] = o[:, TPB : 2 * TPB].T
    return out


# revision 33
# speedup vs baseline: 1.1023x; 1.0347x over previous
"""Distributed Trainium2 Bass kernel for nn_Attention (LN + fused QKV + RoPE +
MHA-with-in-proj + out-proj), SPMD over 8 NeuronCores.

Sharding: both batches sequence-sharded across 8 cores; core c owns rows
[256c, 256c+256) of batch 0 AND batch 1 (512 tokens/core). Projections run on
the mixed 512-token block; attention runs per batch (256 queries x 2048 keys).
K-heads and V rows are exchanged via bf16 AllGathers on the (serial) CC
stream, ordered AG(kh pairs 0-3) -> AG(vh) -> AG(kh pairs 4-7) so the vh
gather lands while the first head-pairs' softmax runs.

Design notes:
 - bf16 everywhere on the matmul path, f32 PSUM. rel err ~1e-2 (budget 2e-2).
 - LayerNorm folded into the qkv projections algebraically:
     qkv = rstd*(W.T x + (-s1) (x) murstd + b1 (x) sd),  s1 = colsum(W)
   one K=2 correction matmul per chunk, one DVE mul epilogue. Projections
   consume raw x, so nothing waits on the LN reduction.
 - PE chain order: kproj, rope-k(in place), k-inproj, vproj, vh, qproj,
   rope-q, q-inproj, attention, out-proj per batch.
 - gpsimd queue carries ONLY collectives (it head-blocks on their waits);
   all broadcasts are 0-partition-stride DMAs, all small moves DMA/DVE.
 - Attention: scores [keys, q] per (b, head); the two heads of a pair run as
   concurrent PE row-group matmuls (K=64 at partitions 0:64 / 64:128). exp on
   ACT (the true critical engine, ~150us); mask applied multiplicatively
   (exp(mask) precomputed). AV appends a ones-column producing the softmax
   denominator at partition 64; reciprocal_approx_fast + DMA-broadcast.
 - The first 4 (b0, hp) units run "scores+exp only" with their attention
   weights staged in SBUF, so ACT keeps running while AG(vh) is in flight;
   their AV matmuls replay from the staging once vh arrives.
"""

import numpy as np

import concourse.bass as bass
import concourse.tile as tile
from concourse import bacc, mybir
from concourse.bass_utils import run_bass_kernel_spmd

B, S, D = 2, 2048, 1024
H, HD = 16, 64
NCORES = 8
TPB = 256  # tokens per core per batch
T = 2 * TPB  # tokens per core
EPS = 1e-5
THETA = 10000.0
P = 128
F32 = mybir.dt.float32
BF16 = mybir.dt.bfloat16
Copy = mybir.ActivationFunctionType.Copy
Ident = mybir.ActivationFunctionType.Identity
Exp = mybir.ActivationFunctionType.Exp
Sqrt = mybir.ActivationFunctionType.Sqrt
MUL = mybir.AluOpType.mult
ADD = mybir.AluOpType.add
SUB = mybir.AluOpType.subtract

TRACE = False  # test.py flips this for profiling runs

N_STAGED = 4  # (b=0, hp<4) units whose attn weights are staged pre-AG2

_cached = {}


def _bcast_ap(src, nparts):
    """0-partition-stride AP replicating src's single partition nparts ways."""
    return bass.AP(
        tensor=src.tensor, offset=src.offset, ap=[[0, nparts]] + src.ap[1:]
    )


def _build_module():
    nc = bacc.Bacc(None, target_bir_lowering=False)

    xT = nc.declare_dram_parameter("xT", [D, T], BF16, isOutput=False)
    maskT = nc.declare_dram_parameter("maskT", [S, T], BF16, isOutput=False)
    cosT = nc.declare_dram_parameter("cosT", [D // 2, T], BF16, isOutput=False)
    sinT = nc.declare_dram_parameter("sinT", [D // 2, T], BF16, isOutput=False)
    w1qkT = nc.declare_dram_parameter("w1qkT", [D, 2 * D], BF16, isOutput=False)
    w1vT = nc.declare_dram_parameter("w1vT", [D, D], BF16, isOutput=False)
    c1qk = nc.declare_dram_parameter("c1qk", [2, 2 * D], BF16, isOutput=False)
    c1v = nc.declare_dram_parameter("c1v", [2, D], BF16, isOutput=False)
    w2T = nc.declare_dram_parameter("w2T", [D, 2 * D], BF16, isOutput=False)
    b2 = nc.declare_dram_parameter("b2", [1, 2 * D], BF16, isOutput=False)
    wvT = nc.declare_dram_parameter("wvT", [D, D], BF16, isOutput=False)
    bvr = nc.declare_dram_parameter("bvr", [1, D], BF16, isOutput=False)
    owT = nc.declare_dram_parameter("owT", [D, D], BF16, isOutput=False)
    outb = nc.declare_dram_parameter("outb", [1, D], BF16, isOutput=False)
    outT = nc.declare_dram_parameter("outT", [D, T], F32, isOutput=True)

    RG = [list(range(NCORES))]

    with tile.TileContext(nc) as tc:
        with (
            tc.tile_pool(name="persist", bufs=1) as persist,
            tc.tile_pool(name="dram", bufs=1, space="DRAM") as dram,
        ):
            qhT = persist.tile([P, 8, T], BF16)  # head-pair-major q heads
            avT = persist.tile([P, 8, T], BF16)  # attention out, feature-major
            expm = persist.tile([P, 16, T], BF16)  # exp(mask), key-major
            c1qk_sb = persist.tile([2, 2 * D], BF16)
            c1v_sb = persist.tile([2, D], BF16)
            b2_sb = persist.tile([1, 2 * D], BF16)
            bvr_sb = persist.tile([1, D], BF16)
            outb_sb = persist.tile([1, D], BF16)
            ones_col = persist.tile([P, 1], BF16)
            ones_row = persist.tile([1, T], BF16)
            eps_sb = persist.tile([1, 1], F32)
            corr_rhs = persist.tile([2, T], BF16)  # row0=murstd row1=sd
            rstd_b = persist.tile([P, T], F32)

            ag1a_in = dram.tile([D // 2, T], BF16)
            ag1a_out = dram.tile([NCORES * D // 2, T], BF16, addr_space="Shared")
            ag1b_in = dram.tile([D // 2, T], BF16)
            ag1b_out = dram.tile([NCORES * D // 2, T], BF16, addr_space="Shared")
            ag2_in = dram.tile([T, D], BF16)
            ag2_out = dram.tile([NCORES * T, D], BF16, addr_space="Shared")
            bcd = dram.tile([2, T], BF16)  # DRAM bounce for corr_rhs rows
            rsd = dram.tile([1, T], F32)  # DRAM bounce for rstd broadcast

            with (
                tc.tile_pool(name="xpool", bufs=1) as xpool,
                tc.tile_pool(name="wpool", bufs=3) as wpool,
            ):
                # x first: it gates LN stats AND every projection matmul
                xfull = xpool.tile([P, 8, T], BF16)
                nc.sync.dma_start(xfull[:], xT.rearrange("(ko p) t -> p ko t", p=P))
                cos_sb = xpool.tile([P, 4, T], BF16)
                sin_sb = xpool.tile([P, 4, T], BF16)
                nc.sync.dma_start(cos_sb[:], cosT.rearrange("(c p) t -> p c t", p=P))
                nc.sync.dma_start(sin_sb[:], sinT.rearrange("(c p) t -> p c t", p=P))
                nc.sync.dma_start(c1qk_sb[:], c1qk[:])
                nc.sync.dma_start(c1v_sb[:], c1v[:])
                nc.sync.dma_start(b2_sb[:], b2[:])
                nc.sync.dma_start(bvr_sb[:], bvr[:])
                nc.sync.dma_start(outb_sb[:], outb[:])
                nc.vector.memset(ones_col[:], 1.0)
                nc.vector.memset(ones_row[:], 1.0)
                nc.vector.memset(eps_sb[:], EPS)

                # ---- LN statistics (sum / sum-of-squares via PE) ----
                with (
                    tc.tile_pool(name="sqp", bufs=2) as sqp,
                    tc.tile_pool(name="lnrows", bufs=1) as lnrows,
                    tc.tile_pool(name="psLN", bufs=2, space="PSUM") as psLN,
                ):
                    pt_s = psLN.tile([1, T], F32, tag="s")
                    pt_q = psLN.tile([1, T], F32, tag="q")
                    for ko in range(8):
                        sq = sqp.tile([P, T], BF16, tag="sq")
                        nc.vector.tensor_tensor(
                            sq[:], xfull[:, ko, :], xfull[:, ko, :], MUL
                        )
                        nc.tensor.matmul(
                            pt_s[0:1, :], ones_col[:], xfull[:, ko, :],
                            start=(ko == 0), stop=(ko == 7),
                        )
                        nc.tensor.matmul(
                            pt_q[0:1, :], ones_col[:], sq[:],
                            start=(ko == 0), stop=(ko == 7),
                        )
                    mu = lnrows.tile([1, T], F32)
                    msq = lnrows.tile([1, T], F32)
                    nc.vector.tensor_scalar_mul(mu[:], pt_s[0:1, :], 1.0 / D)
                    nc.vector.tensor_scalar_mul(msq[:], pt_q[0:1, :], 1.0 / D)
                    var = lnrows.tile([1, T], F32)
                    nc.vector.tensor_tensor(var[:], mu[:], mu[:], MUL)
                    nc.vector.tensor_tensor(var[:], msq[:], var[:], SUB)
                    sd = lnrows.tile([1, T], F32)
                    nc.scalar.activation(
                        out=sd[:], in_=var[:], func=Sqrt, bias=eps_sb[:]
                    )
                    rstd = lnrows.tile([1, T], F32)
                    nc.vector.reciprocal_approx_fast(out=rstd[:], in_=sd[:])
                    murstd = lnrows.tile([1, T], F32)
                    nc.vector.tensor_tensor(murstd[:], mu[:], rstd[:], MUL)
                    sdb = lnrows.tile([1, T], BF16)
                    nc.vector.tensor_copy(sdb[:], sd[:])
                    murb = lnrows.tile([1, T], BF16)
                    nc.vector.tensor_copy(murb[:], murstd[:])
                    # partition placement / broadcast via DRAM (0-stride
                    # partition APs are only legal on DRAM sources)
                    nc.sync.dma_start(bcd[0:1, :], murb[:])
                    nc.sync.dma_start(bcd[1:2, :], sdb[:])
                    nc.sync.dma_start(corr_rhs[0:2, :], bcd[0:2, :])
                    nc.sync.dma_start(rsd[0:1, :], rstd[:])
                    nc.sync.dma_start(rstd_b[:], _bcast_ap(rsd[0:1, :], P))

                w1view = w1qkT.rearrange("(ko p) j -> p ko j", p=P)
                w1vview = w1vT.rearrange("(ko p) j -> p ko j", p=P)
                w2view = w2T.rearrange("(ko p) j -> p ko j", p=P)
                owview = owT.rearrange("(ko p) j -> p ko j", p=P)

                with (
                    tc.tile_pool(name="qk", bufs=2) as qkp,
                    tc.tile_pool(name="rtmp", bufs=2) as rtmp,
                    tc.tile_pool(name="stage", bufs=2) as stagep,
                    tc.tile_pool(name="psA", bufs=3, space="PSUM") as psA,
                ):

                    def project_ln(dst, dst_ko, wview, jcol, corr_sb):
                        """dst[:,dst_ko,:] = rstd*(W.T x + corr.T [murstd; sd])."""
                        wt = wpool.tile([P, 8, P], BF16, tag="w")
                        nc.sync.dma_start(wt[:], wview[:, :, jcol : jcol + P])
                        pt = psA.tile([P, T], F32, tag="proj")
                        for ko in range(8):
                            nc.tensor.matmul(
                                pt[:], wt[:, ko, :], xfull[:, ko, :],
                                start=(ko == 0), stop=False,
                            )
                        nc.tensor.matmul(
                            pt[:], corr_sb[0:2, jcol : jcol + P], corr_rhs[0:2, :],
                            start=False, stop=True,
                        )
                        nc.vector.tensor_tensor(
                            dst[:, dst_ko, :], pt[:], rstd_b[:], MUL
                        )

                    def project_plain(dst, dst_ko, wview, jcol, bias_sb, bofs, rhs):
                        """dst[:,dst_ko,:] = W.T rhs + bias."""
                        wt = wpool.tile([P, 8, P], BF16, tag="w")
                        nc.sync.dma_start(wt[:], wview[:, :, jcol : jcol + P])
                        pt = psA.tile([P, T], F32, tag="proj")
                        for ko in range(8):
                            nc.tensor.matmul(
                                pt[:], wt[:, ko, :], rhs[:, ko, :],
                                start=(ko == 0), stop=False,
                            )
                        nc.tensor.matmul(
                            pt[:], bias_sb[0:1, bofs : bofs + P], ones_row[:],
                            start=False, stop=True,
                        )
                        nc.vector.tensor_copy(dst[:, dst_ko, :], pt[:])

                    def rope_inplace(src):
                        # src[c], src[4+c] <- rotated pair (in place)
                        for c in range(4):
                            x1 = src[:, c, :]
                            x2 = src[:, 4 + c, :]
                            ta = rtmp.tile([P, T], BF16, tag="ra")
                            tb = rtmp.tile([P, T], BF16, tag="rb")
                            tc2 = rtmp.tile([P, T], BF16, tag="ra")
                            td = rtmp.tile([P, T], BF16, tag="rb")
                            nc.vector.tensor_tensor(ta[:], x1, cos_sb[:, c, :], MUL)
                            nc.vector.tensor_tensor(tb[:], x2, sin_sb[:, c, :], MUL)
                            nc.vector.tensor_tensor(tc2[:], x2, cos_sb[:, c, :], MUL)
                            nc.vector.tensor_tensor(td[:], x1, sin_sb[:, c, :], MUL)
                            nc.vector.tensor_tensor(x1, ta[:], tb[:], SUB)
                            nc.vector.tensor_tensor(x2, tc2[:], td[:], ADD)

                    # ---- k chain ----
                    kT = qkp.tile([P, 8, T], BF16, tag="qk")
                    for jm in range(8):
                        project_ln(kT, jm, w1view, D + P * jm, c1qk_sb)
                    rope_inplace(kT)
                    khT = stagep.tile([P, 8, T], BF16, tag="stage")
                    for jm in range(8):
                        project_plain(khT, jm, w2view, D + P * jm, b2_sb, D + P * jm, kT)
                        if jm == 3:
                            nc.sync.dma_start(
                                ag1a_in.rearrange("(ko p) t -> p ko t", p=P),
                                khT[:, 0:4, :],
                            )
                            cc_a1a = nc.gpsimd.collective_compute(
                                "AllGather", mybir.AluOpType.bypass,
                                ins=[ag1a_in.opt()], outs=[ag1a_out.opt()],
                                replica_groups=RG,
                            )

                    # exp(mask): emitted here so its DMAs sit behind the k-chain
                    # weight loads; ACT does these while PE projects
                    with tc.tile_pool(name="mload", bufs=2) as mload:
                        mview = maskT.rearrange("(jc p) t -> p jc t", p=P)
                        for g in range(4):
                            mt = mload.tile([P, 4, T], BF16)
                            nc.sync.dma_start(mt[:], mview[:, 4 * g : 4 * g + 4, :])
                            nc.scalar.activation(
                                out=expm[:, 4 * g : 4 * g + 4, :], in_=mt[:],
                                func=Exp,
                            )

                    # ---- v chain (vh AG goes second on the CC stream) ----
                    vT = qkp.tile([P, 8, T], BF16, tag="qk")
                    for jm in range(8):
                        project_ln(vT, jm, w1vview, P * jm, c1v_sb)
                    with tc.tile_pool(name="wvp", bufs=2) as wvp:
                        vh_bf = stagep.tile([P, 4, D], BF16, tag="stage")
                        wvview = wvT.rearrange("(ko p) n -> p ko n", p=P)
                        for nh in range(4):
                            wv_rhs = wvp.tile([P, 8, 256], BF16)
                            nc.sync.dma_start(
                                wv_rhs[:], wvview[:, :, 256 * nh : 256 * nh + 256]
                            )
                            for tm in range(4):
                                pt = psA.tile([P, T], F32, tag="proj")
                                for ko in range(8):
                                    nc.tensor.matmul(
                                        pt[:, 0:256],
                                        vT[:, ko, P * tm : P * tm + P],
                                        wv_rhs[:, ko, :],
                                        start=(ko == 0), stop=False,
                                    )
                                nc.tensor.matmul(
                                    pt[:, 0:256],
                                    ones_row[0:1, 0:P],
                                    bvr_sb[0:1, 256 * nh : 256 * nh + 256],
                                    start=False, stop=True,
                                )
                                nc.vector.tensor_copy(
                                    vh_bf[:, tm, 256 * nh : 256 * nh + 256],
                                    pt[:, 0:256],
                                )
                        nc.sync.dma_start(
                            ag2_in.rearrange("(tm p) n -> p tm n", p=P),
                            vh_bf[:],
                        )
                        cc_ag2 = nc.gpsimd.collective_compute(
                            "AllGather", mybir.AluOpType.bypass,
                            ins=[ag2_in.opt()], outs=[ag2_out.opt()],
                            replica_groups=RG,
                        )

                    # ---- q chain ----
                    qT = qkp.tile([P, 8, T], BF16, tag="qk")
                    for jm in range(8):
                        project_ln(qT, jm, w1view, P * jm, c1qk_sb)
                    rope_inplace(qT)
                    for jm in range(8):
                        project_plain(qhT, jm, w2view, P * jm, b2_sb, P * jm, qT)

                    # kh pairs 4-7 gather last (not needed until ~exp midpoint);
                    # force it behind AG2 on the serial CC stream
                    nc.sync.dma_start(
                        ag1b_in.rearrange("(ko p) t -> p ko t", p=P),
                        khT[:, 4:8, :],
                    )
                    cc_a1b = nc.gpsimd.collective_compute(
                        "AllGather", mybir.AluOpType.bypass,
                        ins=[ag1b_in.opt()], outs=[ag1b_out.opt()],
                        replica_groups=RG,
                    )

                # ---- attention ----
                # ag1{a,b}_out rows: 512*r + 128*hp' + 64*sub + hd
                # ag2_out rows: 512*r + 256*b + tok ; cols 64*h + hd
                kviewA = ag1a_out.rearrange(
                    "(r hp sub hd) t -> hp (sub hd) r t", hp=4, sub=2, hd=HD
                )
                kviewB = ag1b_out.rearrange(
                    "(r hp sub hd) t -> hp (sub hd) r t", hp=4, sub=2, hd=HD
                )
                vview = ag2_out.rearrange(
                    "(r b2 half p) f -> b2 half p r f", b2=2, half=2, p=P
                )
                with (
                    tc.tile_pool(name="kload", bufs=2) as kload,
                    tc.tile_pool(name="vload", bufs=2) as vload,
                    tc.tile_pool(name="aep", bufs=4) as aep,
                    tc.tile_pool(name="atp", bufs=8 * N_STAGED + 2) as atp,
                    tc.tile_pool(name="nrm", bufs=2) as nrm,
                    tc.tile_pool(name="oc", bufs=2) as ocp,
                    tc.tile_pool(name="psS", bufs=2, space="PSUM") as psS,
                    tc.tile_pool(name="psV", bufs=2, space="PSUM") as psV,
                    tc.tile_pool(name="psD", bufs=1, space="PSUM") as psD,
                ):

                    def load_kp(b, hp):
                        kview = kviewA if hp < 4 else kviewB
                        kp = kload.tile([P, 8, TPB], BF16, tag="kp")
                        nc.sync.dma_start(
                            kp[:], kview[hp % 4][:, :, TPB * b : TPB * b + TPB]
                        )
                        return kp

                    def load_vh(b, hp):
                        vh_sb = vload.tile([P, 8, 2, 2, HD + 1], BF16, tag="vh")
                        for half in range(2):
                            for sub in range(2):
                                c0 = P * hp + HD * sub
                                nc.sync.dma_start(
                                    vh_sb[:, :, half, sub, 0:HD],
                                    vview[b][half][:, :, c0 : c0 + HD],
                                )
                        nc.vector.memset(vh_sb[:, :, :, :, HD : HD + 1], 1.0)
                        return vh_sb

                    def scores_pair(b, hp, kp):
                        """scores+exp+mask for both heads; returns 4x2 attnT."""
                        ats = []
                        for g in range(4):
                            s_pt0 = psS.tile([P, 4, TPB], F32, tag="s")
                            s_pt1 = psS.tile([P, 4, TPB], F32, tag="s")
                            s_pt = [s_pt0, s_pt1]
                            for u in range(4):
                                jc = 4 * g + u
                                r, half = jc // 2, jc % 2
                                for sub in range(2):
                                    h0 = HD * sub
                                    nc.tensor.matmul(
                                        s_pt[sub][:, u, :],
                                        kp[h0 : h0 + HD, r, P * half : P * half + P],
                                        qhT[h0 : h0 + HD, hp, TPB * b : TPB * b + TPB],
                                        start=True, stop=True,
                                    )
                            for sub in range(2):
                                attnE = aep.tile([P, 4, TPB], BF16, tag="ae")
                                nc.scalar.activation(
                                    out=attnE[:], in_=s_pt[sub][:], func=Exp
                                )
                                attnT = atp.tile([P, 4, TPB], BF16, tag="at")
                                nc.vector.tensor_tensor(
                                    attnT[:],
                                    attnE[:],
                                    expm[:, 4 * g : 4 * g + 4, TPB * b : TPB * b + TPB],
                                    MUL,
                                )
                                ats.append((g, sub, attnT))
                        return ats

                    def av_pair(b, hp, vh_sb, ats):
                        av_pt0 = psV.tile([P, TPB], F32, tag="av")
                        av_pt1 = psV.tile([P, TPB], F32, tag="av")
                        av_pt = [av_pt0, av_pt1]
                        for g, sub, attnT in ats:
                            for u in range(4):
                                jc = 4 * g + u
                                nc.tensor.matmul(
                                    av_pt[sub][0 : HD + 1, :],
                                    vh_sb[:, jc // 2, jc % 2, sub, 0 : HD + 1],
                                    attnT[:, u, :],
                                    start=(g == 0 and u == 0),
                                    stop=(g == 3 and u == 3),
                                )
                        norm_pair(b, hp, av_pt)

                    def norm_pair(b, hp, av_pt):
                        for sub in range(2):
                            avs = nrm.tile([P, TPB], F32, tag="avs")
                            nc.vector.tensor_copy(
                                avs[0 : HD + 1, :], av_pt[sub][0 : HD + 1, :]
                            )
                            drow = nrm.tile([1, TPB], F32, tag="dr")
                            nc.sync.dma_start(drow[:], avs[HD : HD + 1, :])
                            rrow = nrm.tile([1, TPB], F32, tag="rr")
                            nc.vector.reciprocal_approx_fast(
                                out=rrow[:], in_=drow[:]
                            )
                            rb = nrm.tile([HD, TPB], F32, tag="rbt")
                            nc.gpsimd.partition_broadcast(rb[:], rrow[:])
                            if sub == 0:
                                nc.vector.tensor_tensor(
                                    avT[0:HD, hp, TPB * b : TPB * b + TPB],
                                    avs[0:HD, :], rb[:], MUL,
                                )
                            else:
                                avn = nrm.tile([HD, TPB], BF16, tag="avn")
                                nc.vector.tensor_tensor(
                                    avn[:], avs[0:HD, :], rb[:], MUL
                                )
                                nc.sync.dma_start(
                                    avT[HD:P, hp, TPB * b : TPB * b + TPB],
                                    avn[:],
                                )

                    def outproj(b, oms):
                        for om in oms:
                            owt = wpool.tile([P, 8, P], BF16, tag="w")
                            nc.sync.dma_start(
                                owt[:], owview[:, :, P * om : P * om + P]
                            )
                            pt = psV.tile([P, TPB], F32, tag="av")
                            for ko in range(8):
                                nc.tensor.matmul(
                                    pt[:],
                                    owt[:, ko, :],
                                    avT[:, ko, TPB * b : TPB * b + TPB],
                                    start=(ko == 0), stop=False,
                                )
                            nc.tensor.matmul(
                                pt[:],
                                outb_sb[0:1, P * om : P * om + P],
                                ones_row[0:1, 0:TPB],
                                start=False, stop=True,
                            )
                            oc = ocp.tile([P, TPB], F32, tag="oc")
                            nc.vector.tensor_copy(oc[:], pt[:])
                            nc.sync.dma_start(
                                outT.rearrange("(ko p) t -> p ko t", p=P)[
                                    :, om, TPB * b : TPB * b + TPB
                                ],
                                oc[:],
                            )

                    # Phase A: stage scores/exp for (b0, hp<N_STAGED) while the
                    # vh AllGather is still in flight
                    staged = []
                    for hp in range(N_STAGED):
                        kp = load_kp(0, hp)
                        staged.append(scores_pair(0, hp, kp))
                    # Phase B: replay their AVs (unblocks when AG2 lands)
                    for hp in range(N_STAGED):
                        vh_sb = load_vh(0, hp)
                        av_pair(0, hp, vh_sb, staged[hp])
                    staged = None
                    # Phase C: remaining units, interleaved normally.
                    # (b1, hp<4) next (kh already gathered), then hp>=4.
                    order = (
                        [(1, hp) for hp in range(4)]
                        + [(0, hp) for hp in range(N_STAGED, 4)]
                        + [(0, hp) for hp in range(4, 8)]
                        + [(1, hp) for hp in range(4, 8)]
                    )
                    done_b0 = False
                    for b, hp in order:
                        kp = load_kp(b, hp)
                        vh_sb = load_vh(b, hp)
                        # interleaved scores/exp/AV with filler matmuls into an
                        # unused av_pt row: keeps the PE activity window gapless
                        # so the HAM clock-gate stays at full rate
                        av_pt0 = psV.tile([P, TPB], F32, tag="av")
                        av_pt1 = psV.tile([P, TPB], F32, tag="av")
                        av_pt = [av_pt0, av_pt1]
                        dum_pt = psD.tile([1, T], F32, tag="dum")
                        for g in range(4):
                            s_pt0 = psS.tile([P, 4, TPB], F32, tag="s")
                            s_pt1 = psS.tile([P, 4, TPB], F32, tag="s")
                            s_pt = [s_pt0, s_pt1]
                            for u in range(4):
                                jc = 4 * g + u
                                r, half = jc // 2, jc % 2
                                for sub in range(2):
                                    h0 = HD * sub
                                    nc.tensor.matmul(
                                        s_pt[sub][:, u, :],
                                        kp[h0 : h0 + HD, r, P * half : P * half + P],
                                        qhT[h0 : h0 + HD, hp, TPB * b : TPB * b + TPB],
                                        start=True, stop=True,
                                    )
                            for f in range(4):
                                nc.tensor.matmul(
                                    dum_pt[0:1, :],
                                    ones_col[:],
                                    expm[:, f, :],
                                    start=True, stop=True,
                                    skip_group_check=True,
                                )
                            for sub in range(2):
                                attnE = aep.tile([P, 4, TPB], BF16, tag="ae")
                                nc.scalar.activation(
                                    out=attnE[:], in_=s_pt[sub][:], func=Exp
                                )
                                attnT = atp.tile([P, 4, TPB], BF16, tag="at")
                                nc.vector.tensor_tensor(
                                    attnT[:],
                                    attnE[:],
                                    expm[:, 4 * g : 4 * g + 4, TPB * b : TPB * b + TPB],
                                    MUL,
                                )
                                for u in range(4):
                                    jc = 4 * g + u
                                    nc.tensor.matmul(
                                        av_pt[sub][0 : HD + 1, :],
                                        vh_sb[:, jc // 2, jc % 2, sub, 0 : HD + 1],
                                        attnT[:, u, :],
                                        start=(g == 0 and u == 0),
                                        stop=(g == 3 and u == 3),
                                    )
                        norm_pair(b, hp, av_pt)
                        if b == 0 and hp == 7:
                            done_b0 = True
                        # spread batch-0's out-projection over the last four
                        # units so it never starves ACT of score work
                        if b == 1 and hp >= 4:
                            assert done_b0
                            outproj(0, [2 * (hp - 4), 2 * (hp - 4) + 1])
                    outproj(1, range(8))

    nc.finalize()
    return nc


def _bf16(x):
    x = np.ascontiguousarray(np.asarray(x, np.float32))
    u = x.view(np.uint32)
    r = ((u >> 16) & 1).astype(np.uint32)
    return ((u + 0x7FFF + r) & 0xFFFF0000).view(np.float32)


def _host_prep(x, mask, ln_g, ln_b, w_qkv, b_qkv, in_w, in_b, out_w, out_b):
    f32 = np.float32
    import ml_dtypes

    def to_bf(a):
        return np.asarray(a, np.float32).astype(ml_dtypes.bfloat16)

    perm = np.concatenate([np.arange(0, D, 2), np.arange(1, D, 2)])
    W1 = (w_qkv * ln_g[None, :]).astype(f32)
    b1 = (b_qkv + w_qkv @ ln_b).astype(f32)
    W1q, W1k, W1v = W1[0:D], W1[D : 2 * D], W1[2 * D :]
    b1q, b1k, b1v = b1[0:D], b1[D : 2 * D], b1[2 * D :]
    w1qkT = _bf16(np.concatenate([W1q[perm], W1k[perm]], axis=0).T)  # (D,2D)
    c1qk = np.stack(
        [-w1qkT.sum(axis=0), np.concatenate([b1q[perm], b1k[perm]])]
    ).astype(f32)  # (2, 2D): row0=-s1 row1=b1
    w1vT = _bf16(W1v.T)
    c1v = np.stack([-w1vT.sum(axis=0), b1v]).astype(f32)

    wq, wk, wv = in_w[0:D], in_w[D : 2 * D], in_w[2 * D :]
    bq, bk, bv = in_b[0:D], in_b[D : 2 * D], in_b[2 * D :]
    SC = 1.0 / np.sqrt(HD)
    w2q = np.ascontiguousarray((wq * SC).T[perm])
    w2k = np.ascontiguousarray(wk.T[perm])
    w2T = np.concatenate([w2q, w2k], axis=1).astype(f32)
    b2r = np.concatenate([bq * SC, bk]).reshape(1, 2 * D).astype(f32)
    wvT2 = np.ascontiguousarray(wv.T).astype(f32)
    bvr = bv.reshape(1, D).astype(f32)
    owT = np.ascontiguousarray(out_w.T).astype(f32)
    outbr = out_b.reshape(1, D).astype(f32)

    inv_freq = 1.0 / (THETA ** (np.arange(0, D, 2, dtype=np.float64) / D))

    shared = dict(
        w1qkT=to_bf(w1qkT), w1vT=to_bf(w1vT), c1qk=to_bf(c1qk), c1v=to_bf(c1v),
        w2T=to_bf(w2T), b2=to_bf(b2r), wvT=to_bf(wvT2), bvr=to_bf(bvr),
        owT=to_bf(owT), outb=to_bf(outbr),
    )
    in_maps = []
    for c in range(NCORES):
        rows = slice(TPB * c, TPB * c + TPB)
        xc = np.ascontiguousarray(
            np.concatenate([x[0, rows], x[1, rows]], axis=0).T
        )
        mc = np.ascontiguousarray(
            np.concatenate([mask[0, rows].T, mask[1, rows].T], axis=1)
        )
        pos = np.arange(TPB * c, TPB * c + TPB, dtype=np.float64)
        ang = inv_freq[:, None] * pos[None, :]  # (512, 256)
        cosc = np.cos(ang)
        sinc = np.sin(ang)
        m = dict(shared)
        m["xT"] = to_bf(xc)
        m["maskT"] = to_bf(mc)
        m["cosT"] = to_bf(np.concatenate([cosc, cosc], axis=1))
        m["sinT"] = to_bf(np.concatenate([sinc, sinc], axis=1))
        in_maps.append(m)
    return in_maps


def kernel(**inputs):
    if "nc" not in _cached:
        _cached["nc"] = _build_module()
    nc = _cached["nc"]
    in_maps = _host_prep(**inputs)
    res = run_bass_kernel_spmd(nc, in_maps, list(range(NCORES)), trace=TRACE)
    _cached["last_result"] = res
    out = np.empty((B, S, D), dtype=np.float32)
    for c in range(NCORES):
        o = res.results[c]["outT"]  # (D, 512)
        rows = slice(TPB * c, TPB * c + TPB)
        out[0, rows] = o[:, 0:TPB].T
        out[1, rows] = o[:, TPB : 2 * TPB].T
    return out


# revision 35
# speedup vs baseline: 1.1253x; 1.0209x over previous
"""Distributed Trainium2 Bass kernel for nn_Attention (LN + fused QKV + RoPE +
MHA-with-in-proj + out-proj), SPMD over 8 NeuronCores.

Sharding: both batches sequence-sharded across 8 cores; core c owns rows
[256c, 256c+256) of batch 0 AND batch 1 (512 tokens/core). Projections run on
the mixed 512-token block; attention runs per batch (256 queries x 2048 keys).
K-heads and V rows are exchanged via bf16 AllGathers on the (serial) CC
stream, ordered AG(kh pairs 0-3) -> AG(vh) -> AG(kh pairs 4-7) so the vh
gather lands while the first head-pairs' softmax runs.

Design notes:
 - bf16 everywhere on the matmul path, f32 PSUM. rel err ~1e-2 (budget 2e-2).
 - LayerNorm folded into the qkv projections algebraically:
     qkv = rstd*(W.T x + (-s1) (x) murstd + b1 (x) sd),  s1 = colsum(W)
   one K=2 correction matmul per chunk, one DVE mul epilogue. Projections
   consume raw x, so nothing waits on the LN reduction.
 - PE chain order: kproj, rope-k(in place), k-inproj, vproj, vh, qproj,
   rope-q, q-inproj, attention, out-proj per batch.
 - gpsimd queue carries ONLY collectives (it head-blocks on their waits);
   all broadcasts are 0-partition-stride DMAs, all small moves DMA/DVE.
 - Attention: scores [keys, q] per (b, head); the two heads of a pair run as
   concurrent PE row-group matmuls (K=64 at partitions 0:64 / 64:128). exp on
   ACT (the true critical engine, ~150us); mask applied multiplicatively
   (exp(mask) precomputed). AV appends a ones-column producing the softmax
   denominator at partition 64; reciprocal_approx_fast + DMA-broadcast.
 - The first 4 (b0, hp) units run "scores+exp only" with their attention
   weights staged in SBUF, so ACT keeps running while AG(vh) is in flight;
   their AV matmuls replay from the staging once vh arrives.
"""

import numpy as np

import concourse.bass as bass
import concourse.tile as tile
from concourse import bacc, mybir
from concourse.bass_utils import run_bass_kernel_spmd

B, S, D = 2, 2048, 1024
H, HD = 16, 64
NCORES = 8
TPB = 256  # tokens per core per batch
T = 2 * TPB  # tokens per core
EPS = 1e-5
THETA = 10000.0
P = 128
F32 = mybir.dt.float32
BF16 = mybir.dt.bfloat16
Copy = mybir.ActivationFunctionType.Copy
Ident = mybir.ActivationFunctionType.Identity
Exp = mybir.ActivationFunctionType.Exp
Sqrt = mybir.ActivationFunctionType.Sqrt
MUL = mybir.AluOpType.mult
ADD = mybir.AluOpType.add
SUB = mybir.AluOpType.subtract

TRACE = False  # test.py flips this for profiling runs

N_STAGED = 4  # (b=0, hp<4) units whose attn weights are staged pre-AG2

_cached = {}


def _bcast_ap(src, nparts):
    """0-partition-stride AP replicating src's single partition nparts ways."""
    return bass.AP(
        tensor=src.tensor, offset=src.offset, ap=[[0, nparts]] + src.ap[1:]
    )


def _build_module():
    nc = bacc.Bacc(None, target_bir_lowering=False)

    xT = nc.declare_dram_parameter("xT", [D, T], BF16, isOutput=False)
    maskT = nc.declare_dram_parameter("maskT", [S, T], BF16, isOutput=False)
    cosT = nc.declare_dram_parameter("cosT", [D // 2, T], BF16, isOutput=False)
    sinT = nc.declare_dram_parameter("sinT", [D // 2, T], BF16, isOutput=False)
    w1qkT = nc.declare_dram_parameter("w1qkT", [D, 2 * D], BF16, isOutput=False)
    w1vT = nc.declare_dram_parameter("w1vT", [D, D], BF16, isOutput=False)
    c1qk = nc.declare_dram_parameter("c1qk", [2, 2 * D], BF16, isOutput=False)
    c1v = nc.declare_dram_parameter("c1v", [2, D], BF16, isOutput=False)
    w2T = nc.declare_dram_parameter("w2T", [D, 2 * D], BF16, isOutput=False)
    b2 = nc.declare_dram_parameter("b2", [1, 2 * D], BF16, isOutput=False)
    wvT = nc.declare_dram_parameter("wvT", [D, D], BF16, isOutput=False)
    bvr = nc.declare_dram_parameter("bvr", [1, D], BF16, isOutput=False)
    owT = nc.declare_dram_parameter("owT", [D, D], BF16, isOutput=False)
    outb = nc.declare_dram_parameter("outb", [1, D], BF16, isOutput=False)
    outT = nc.declare_dram_parameter("outT", [D, T], F32, isOutput=True)

    RG = [list(range(NCORES))]

    with tile.TileContext(nc) as tc:
        with (
            tc.tile_pool(name="persist", bufs=1) as persist,
            tc.tile_pool(name="dram", bufs=1, space="DRAM") as dram,
        ):
            qhT = persist.tile([P, 8, T], BF16)  # head-pair-major q heads
            avT = persist.tile([P, 8, T], BF16)  # attention out, feature-major
            expm = persist.tile([P, 16, T], BF16)  # exp(mask), key-major
            c1qk_sb = persist.tile([2, 2 * D], BF16)
            c1v_sb = persist.tile([2, D], BF16)
            b2_sb = persist.tile([1, 2 * D], BF16)
            bvr_sb = persist.tile([1, D], BF16)
            outb_sb = persist.tile([1, D], BF16)
            ones_col = persist.tile([P, 1], BF16)
            ones_row = persist.tile([1, T], BF16)
            eps_sb = persist.tile([1, 1], F32)
            corr_rhs = persist.tile([2, T], BF16)  # row0=murstd row1=sd
            rstd_b = persist.tile([P, T], F32)

            ag1a_in = dram.tile([D // 2, T], BF16)
            ag1a_out = dram.tile([NCORES * D // 2, T], BF16, addr_space="Shared")
            ag1b_in = dram.tile([D // 2, T], BF16)
            ag1b_out = dram.tile([NCORES * D // 2, T], BF16, addr_space="Shared")
            ag2_in = dram.tile([T, D], BF16)
            ag2_out = dram.tile([NCORES * T, D], BF16, addr_space="Shared")
            bcd = dram.tile([2, T], BF16)  # DRAM bounce for corr_rhs rows
            rsd = dram.tile([1, T], F32)  # DRAM bounce for rstd broadcast

            with (
                tc.tile_pool(name="xpool", bufs=1) as xpool,
                tc.tile_pool(name="wpool", bufs=4) as wpool,
            ):
                # x first: it gates LN stats AND every projection matmul
                xfull = xpool.tile([P, 8, T], BF16)
                nc.sync.dma_start(xfull[:], xT.rearrange("(ko p) t -> p ko t", p=P))
                cos_sb = xpool.tile([P, 4, T], BF16)
                sin_sb = xpool.tile([P, 4, T], BF16)
                nc.sync.dma_start(cos_sb[:], cosT.rearrange("(c p) t -> p c t", p=P))
                nc.sync.dma_start(sin_sb[:], sinT.rearrange("(c p) t -> p c t", p=P))
                nc.sync.dma_start(c1qk_sb[:], c1qk[:])
                nc.sync.dma_start(c1v_sb[:], c1v[:])
                nc.sync.dma_start(b2_sb[:], b2[:])
                nc.sync.dma_start(bvr_sb[:], bvr[:])
                nc.sync.dma_start(outb_sb[:], outb[:])
                nc.vector.memset(ones_col[:], 1.0)
                nc.vector.memset(ones_row[:], 1.0)
                nc.vector.memset(eps_sb[:], EPS)

                # ---- LN statistics (sum / sum-of-squares via PE) ----
                with (
                    tc.tile_pool(name="sqp", bufs=2) as sqp,
                    tc.tile_pool(name="lnrows", bufs=1) as lnrows,
                    tc.tile_pool(name="psLN", bufs=2, space="PSUM") as psLN,
                ):
                    pt_s = psLN.tile([1, T], F32, tag="s")
                    pt_q = psLN.tile([1, T], F32, tag="q")
                    for ko in range(8):
                        sq = sqp.tile([P, T], BF16, tag="sq")
                        nc.vector.tensor_tensor(
                            sq[:], xfull[:, ko, :], xfull[:, ko, :], MUL
                        )
                        nc.tensor.matmul(
                            pt_s[0:1, :], ones_col[:], xfull[:, ko, :],
                            start=(ko == 0), stop=(ko == 7),
                        )
                        nc.tensor.matmul(
                            pt_q[0:1, :], ones_col[:], sq[:],
                            start=(ko == 0), stop=(ko == 7),
                        )
                    mu = lnrows.tile([1, T], F32)
                    msq = lnrows.tile([1, T], F32)
                    nc.vector.tensor_scalar_mul(mu[:], pt_s[0:1, :], 1.0 / D)
                    nc.vector.tensor_scalar_mul(msq[:], pt_q[0:1, :], 1.0 / D)
                    var = lnrows.tile([1, T], F32)
                    nc.vector.tensor_tensor(var[:], mu[:], mu[:], MUL)
                    nc.vector.tensor_tensor(var[:], msq[:], var[:], SUB)
                    sd = lnrows.tile([1, T], F32)
                    nc.scalar.activation(
                        out=sd[:], in_=var[:], func=Sqrt, bias=eps_sb[:]
                    )
                    rstd = lnrows.tile([1, T], F32)
                    nc.vector.reciprocal_approx_fast(out=rstd[:], in_=sd[:])
                    murstd = lnrows.tile([1, T], F32)
                    nc.vector.tensor_tensor(murstd[:], mu[:], rstd[:], MUL)
                    sdb = lnrows.tile([1, T], BF16)
                    nc.vector.tensor_copy(sdb[:], sd[:])
                    murb = lnrows.tile([1, T], BF16)
                    nc.vector.tensor_copy(murb[:], murstd[:])
                    # partition placement / broadcast via DRAM (0-stride
                    # partition APs are only legal on DRAM sources)
                    nc.sync.dma_start(bcd[0:1, :], murb[:])
                    nc.sync.dma_start(bcd[1:2, :], sdb[:])
                    nc.sync.dma_start(corr_rhs[0:2, :], bcd[0:2, :])
                    nc.sync.dma_start(rsd[0:1, :], rstd[:])
                    nc.sync.dma_start(rstd_b[:], _bcast_ap(rsd[0:1, :], P))

                w1view = w1qkT.rearrange("(ko p) j -> p ko j", p=P)
                w1vview = w1vT.rearrange("(ko p) j -> p ko j", p=P)
                w2view = w2T.rearrange("(ko p) j -> p ko j", p=P)
                owview = owT.rearrange("(ko p) j -> p ko j", p=P)

                with (
                    tc.tile_pool(name="qk", bufs=2) as qkp,
                    tc.tile_pool(name="rtmp", bufs=2) as rtmp,
                    tc.tile_pool(name="stage", bufs=2) as stagep,
                    tc.tile_pool(name="psA", bufs=3, space="PSUM") as psA,
                ):

                    def project_ln(dst, dst_ko, wview, jcol, corr_sb):
                        """dst[:,dst_ko,:] = rstd*(W.T x + corr.T [murstd; sd])."""
                        wt = wpool.tile([P, 8, P], BF16, tag="w")
                        nc.sync.dma_start(wt[:], wview[:, :, jcol : jcol + P])
                        pt = psA.tile([P, T], F32, tag="proj")
                        for ko in range(8):
                            nc.tensor.matmul(
                                pt[:], wt[:, ko, :], xfull[:, ko, :],
                                start=(ko == 0), stop=False,
                            )
                        nc.tensor.matmul(
                            pt[:], corr_sb[0:2, jcol : jcol + P], corr_rhs[0:2, :],
                            start=False, stop=True,
                        )
                        nc.vector.tensor_tensor(
                            dst[:, dst_ko, :], pt[:], rstd_b[:], MUL
                        )

                    def project_plain(dst, dst_ko, wview, jcol, bias_sb, bofs, rhs):
                        """dst[:,dst_ko,:] = W.T rhs + bias."""
                        wt = wpool.tile([P, 8, P], BF16, tag="w")
                        nc.sync.dma_start(wt[:], wview[:, :, jcol : jcol + P])
                        pt = psA.tile([P, T], F32, tag="proj")
                        for ko in range(8):
                            nc.tensor.matmul(
                                pt[:], wt[:, ko, :], rhs[:, ko, :],
                                start=(ko == 0), stop=False,
                            )
                        nc.tensor.matmul(
                            pt[:], bias_sb[0:1, bofs : bofs + P], ones_row[:],
                            start=False, stop=True,
                        )
                        nc.vector.tensor_copy(dst[:, dst_ko, :], pt[:])

                    def rope_inplace(src):
                        # src[c], src[4+c] <- rotated pair (in place)
                        for c in range(4):
                            x1 = src[:, c, :]
                            x2 = src[:, 4 + c, :]
                            ta = rtmp.tile([P, T], BF16, tag="ra")
                            tb = rtmp.tile([P, T], BF16, tag="rb")
                            tc2 = rtmp.tile([P, T], BF16, tag="ra")
                            td = rtmp.tile([P, T], BF16, tag="rb")
                            nc.vector.tensor_tensor(ta[:], x1, cos_sb[:, c, :], MUL)
                            nc.vector.tensor_tensor(tb[:], x2, sin_sb[:, c, :], MUL)
                            nc.vector.tensor_tensor(tc2[:], x2, cos_sb[:, c, :], MUL)
                            nc.vector.tensor_tensor(td[:], x1, sin_sb[:, c, :], MUL)
                            nc.vector.tensor_tensor(x1, ta[:], tb[:], SUB)
                            nc.vector.tensor_tensor(x2, tc2[:], td[:], ADD)

                    # ---- k chain ----
                    kT = qkp.tile([P, 8, T], BF16, tag="qk")
                    for jm in range(8):
                        project_ln(kT, jm, w1view, D + P * jm, c1qk_sb)
                    rope_inplace(kT)
                    khT = stagep.tile([P, 8, T], BF16, tag="stage")
                    for jm in range(8):
                        project_plain(khT, jm, w2view, D + P * jm, b2_sb, D + P * jm, kT)
                        if jm == 3:
                            nc.sync.dma_start(
                                ag1a_in.rearrange("(ko p) t -> p ko t", p=P),
                                khT[:, 0:4, :],
                            )
                            cc_a1a = nc.gpsimd.collective_compute(
                                "AllGather", mybir.AluOpType.bypass,
                                ins=[ag1a_in.opt()], outs=[ag1a_out.opt()],
                                replica_groups=RG,
                            )

                    # exp(mask): emitted here so its DMAs sit behind the k-chain
                    # weight loads; ACT does these while PE projects
                    with tc.tile_pool(name="mload", bufs=2) as mload:
                        mview = maskT.rearrange("(jc p) t -> p jc t", p=P)
                        for g in range(4):
                            mt = mload.tile([P, 4, T], BF16)
                            nc.sync.dma_start(mt[:], mview[:, 4 * g : 4 * g + 4, :])
                            nc.scalar.activation(
                                out=expm[:, 4 * g : 4 * g + 4, :], in_=mt[:],
                                func=Exp,
                            )

                    # ---- v chain (vh AG goes second on the CC stream) ----
                    vT = qkp.tile([P, 8, T], BF16, tag="qk")
                    for jm in range(8):
                        project_ln(vT, jm, w1vview, P * jm, c1v_sb)
                    with tc.tile_pool(name="wvp", bufs=2) as wvp:
                        vh_bf = stagep.tile([P, 4, D], BF16, tag="stage")
                        wvview = wvT.rearrange("(ko p) n -> p ko n", p=P)
                        for nh in range(4):
                            wv_rhs = wvp.tile([P, 8, 256], BF16)
                            nc.sync.dma_start(
                                wv_rhs[:], wvview[:, :, 256 * nh : 256 * nh + 256]
                            )
                            for tm in range(4):
                                pt = psA.tile([P, T], F32, tag="proj")
                                for ko in range(8):
                                    nc.tensor.matmul(
                                        pt[:, 0:256],
                                        vT[:, ko, P * tm : P * tm + P],
                                        wv_rhs[:, ko, :],
                                        start=(ko == 0), stop=False,
                                    )
                                nc.tensor.matmul(
                                    pt[:, 0:256],
                                    ones_row[0:1, 0:P],
                                    bvr_sb[0:1, 256 * nh : 256 * nh + 256],
                                    start=False, stop=True,
                                )
                                nc.vector.tensor_copy(
                                    vh_bf[:, tm, 256 * nh : 256 * nh + 256],
                                    pt[:, 0:256],
                                )
                        nc.sync.dma_start(
                            ag2_in.rearrange("(tm p) n -> p tm n", p=P),
                            vh_bf[:],
                        )
                        cc_ag2 = nc.gpsimd.collective_compute(
                            "AllGather", mybir.AluOpType.bypass,
                            ins=[ag2_in.opt()], outs=[ag2_out.opt()],
                            replica_groups=RG,
                        )

                    # ---- q chain ----
                    qT = qkp.tile([P, 8, T], BF16, tag="qk")
                    for jm in range(8):
                        project_ln(qT, jm, w1view, P * jm, c1qk_sb)
                    rope_inplace(qT)
                    for jm in range(8):
                        project_plain(qhT, jm, w2view, P * jm, b2_sb, P * jm, qT)

                    # kh pairs 4-7 gather last (not needed until ~exp midpoint);
                    # force it behind AG2 on the serial CC stream
                    nc.sync.dma_start(
                        ag1b_in.rearrange("(ko p) t -> p ko t", p=P),
                        khT[:, 4:8, :],
                    )
                    cc_a1b = nc.gpsimd.collective_compute(
                        "AllGather", mybir.AluOpType.bypass,
                        ins=[ag1b_in.opt()], outs=[ag1b_out.opt()],
                        replica_groups=RG,
                    )

                # ---- attention ----
                # ag1{a,b}_out rows: 512*r + 128*hp' + 64*sub + hd
                # ag2_out rows: 512*r + 256*b + tok ; cols 64*h + hd
                kviewA = ag1a_out.rearrange(
                    "(r hp sub hd) t -> hp (sub hd) r t", hp=4, sub=2, hd=HD
                )
                kviewB = ag1b_out.rearrange(
                    "(r hp sub hd) t -> hp (sub hd) r t", hp=4, sub=2, hd=HD
                )
                vview = ag2_out.rearrange(
                    "(r b2 half p) f -> b2 half p r f", b2=2, half=2, p=P
                )
                with (
                    tc.tile_pool(name="kload", bufs=3) as kload,
                    tc.tile_pool(name="vload", bufs=3) as vload,
                    tc.tile_pool(name="aep", bufs=4) as aep,
                    tc.tile_pool(name="atp", bufs=8 * N_STAGED + 2) as atp,
                    tc.tile_pool(name="nrm", bufs=2) as nrm,
                    tc.tile_pool(name="oc", bufs=2) as ocp,
                    tc.tile_pool(name="psS", bufs=2, space="PSUM") as psS,
                    tc.tile_pool(name="psV", bufs=2, space="PSUM") as psV,
                    tc.tile_pool(name="psD", bufs=1, space="PSUM") as psD,
                ):

                    def load_kp(b, hp):
                        kview = kviewA if hp < 4 else kviewB
                        kp = kload.tile([P, 8, TPB], BF16, tag="kp")
                        nc.sync.dma_start(
                            kp[:], kview[hp % 4][:, :, TPB * b : TPB * b + TPB]
                        )
                        return kp

                    def load_vh(b, hp):
                        vh_sb = vload.tile([P, 8, 2, 2, HD + 1], BF16, tag="vh")
                        for half in range(2):
                            for sub in range(2):
                                c0 = P * hp + HD * sub
                                nc.sync.dma_start(
                                    vh_sb[:, :, half, sub, 0:HD],
                                    vview[b][half][:, :, c0 : c0 + HD],
                                )
                        nc.vector.memset(vh_sb[:, :, :, :, HD : HD + 1], 1.0)
                        return vh_sb

                    def scores_pair(b, hp, kp):
                        """scores+exp+mask for both heads; returns 4x2 attnT."""
                        ats = []
                        for g in range(4):
                            s_pt0 = psS.tile([P, 4, TPB], F32, tag="s")
                            s_pt1 = psS.tile([P, 4, TPB], F32, tag="s")
                            s_pt = [s_pt0, s_pt1]
                            for u in range(4):
                                jc = 4 * g + u
                                r, half = jc // 2, jc % 2
                                for sub in range(2):
                                    h0 = HD * sub
                                    nc.tensor.matmul(
                                        s_pt[sub][:, u, :],
                                        kp[h0 : h0 + HD, r, P * half : P * half + P],
                                        qhT[h0 : h0 + HD, hp, TPB * b : TPB * b + TPB],
                                        start=True, stop=True,
                                    )
                            for sub in range(2):
                                attnE = aep.tile([P, 4, TPB], BF16, tag="ae")
                                nc.scalar.activation(
                                    out=attnE[:], in_=s_pt[sub][:], func=Exp
                                )
                                attnT = atp.tile([P, 4, TPB], BF16, tag="at")
                                nc.vector.tensor_tensor(
                                    attnT[:],
                                    attnE[:],
                                    expm[:, 4 * g : 4 * g + 4, TPB * b : TPB * b + TPB],
                                    MUL,
                                )
                                ats.append((g, sub, attnT))
                        return ats

                    def av_pair(b, hp, vh_sb, ats):
                        av_pt0 = psV.tile([P, TPB], F32, tag="av")
                        av_pt1 = psV.tile([P, TPB], F32, tag="av")
                        av_pt = [av_pt0, av_pt1]
                        for g, sub, attnT in ats:
                            for u in range(4):
                                jc = 4 * g + u
                                nc.tensor.matmul(
                                    av_pt[sub][0 : HD + 1, :],
                                    vh_sb[:, jc // 2, jc % 2, sub, 0 : HD + 1],
                                    attnT[:, u, :],
                                    start=(g == 0 and u == 0),
                                    stop=(g == 3 and u == 3),
                                )
                        norm_pair(b, hp, av_pt)

                    def norm_pair(b, hp, av_pt):
                        for sub in range(2):
                            avs = nrm.tile([P, TPB], F32, tag="avs")
                            nc.vector.tensor_copy(
                                avs[0 : HD + 1, :], av_pt[sub][0 : HD + 1, :]
                            )
                            drow = nrm.tile([1, TPB], F32, tag="dr")
                            nc.sync.dma_start(drow[:], avs[HD : HD + 1, :])
                            rrow = nrm.tile([1, TPB], F32, tag="rr")
                            nc.vector.reciprocal_approx_fast(
                                out=rrow[:], in_=drow[:]
                            )
                            rb = nrm.tile([HD, TPB], F32, tag="rbt")
                            nc.gpsimd.partition_broadcast(rb[:], rrow[:])
                            if sub == 0:
                                nc.vector.tensor_tensor(
                                    avT[0:HD, hp, TPB * b : TPB * b + TPB],
                                    avs[0:HD, :], rb[:], MUL,
                                )
                            else:
                                avn = nrm.tile([HD, TPB], BF16, tag="avn")
                                nc.vector.tensor_tensor(
                                    avn[:], avs[0:HD, :], rb[:], MUL
                                )
                                nc.sync.dma_start(
                                    avT[HD:P, hp, TPB * b : TPB * b + TPB],
                                    avn[:],
                                )

                    def outproj(b):
                        for om in range(8):
                            owt = wpool.tile([P, 8, P], BF16, tag="w")
                            nc.sync.dma_start(
                                owt[:], owview[:, :, P * om : P * om + P]
                            )
                            pt = psV.tile([P, TPB], F32, tag="av")
                            for ko in range(8):
                                nc.tensor.matmul(
                                    pt[:],
                                    owt[:, ko, :],
                                    avT[:, ko, TPB * b : TPB * b + TPB],
                                    start=(ko == 0), stop=False,
                                )
                            nc.tensor.matmul(
                                pt[:],
                                outb_sb[0:1, P * om : P * om + P],
                                ones_row[0:1, 0:TPB],
                                start=False, stop=True,
                            )
                            oc = ocp.tile([P, TPB], F32, tag="oc")
                            nc.vector.tensor_copy(oc[:], pt[:])
                            nc.sync.dma_start(
                                outT.rearrange("(ko p) t -> p ko t", p=P)[
                                    :, om, TPB * b : TPB * b + TPB
                                ],
                                oc[:],
                            )

                    # Phase A: stage scores/exp for (b0, hp<N_STAGED) while the
                    # vh AllGather is still in flight
                    staged = []
                    for hp in range(N_STAGED):
                        kp = load_kp(0, hp)
                        staged.append(scores_pair(0, hp, kp))
                    # Phase B: replay their AVs (unblocks when AG2 lands)
                    for hp in range(N_STAGED):
                        vh_sb = load_vh(0, hp)
                        av_pair(0, hp, vh_sb, staged[hp])
                    staged = None
                    # Phase C: remaining units, interleaved normally.
                    # (b1, hp<4) next (kh already gathered), then hp>=4.
                    order = (
                        [(1, hp) for hp in range(4)]
                        + [(0, hp) for hp in range(N_STAGED, 4)]
                        + [(0, hp) for hp in range(4, 8)]
                        + [(1, hp) for hp in range(4, 8)]
                    )
                    done_b0 = False
                    for b, hp in order:
                        kp = load_kp(b, hp)
                        vh_sb = load_vh(b, hp)
                        # interleaved scores/exp/AV with filler matmuls into an
                        # unused av_pt row: keeps the PE activity window gapless
                        # so the HAM clock-gate stays at full rate
                        av_pt0 = psV.tile([P, TPB], F32, tag="av")
                        av_pt1 = psV.tile([P, TPB], F32, tag="av")
                        av_pt = [av_pt0, av_pt1]
                        dum_pt = psD.tile([1, T], F32, tag="dum")
                        for g in range(4):
                            s_pt0 = psS.tile([P, 4, TPB], F32, tag="s")
                            s_pt1 = psS.tile([P, 4, TPB], F32, tag="s")
                            s_pt = [s_pt0, s_pt1]
                            for u in range(4):
                                jc = 4 * g + u
                                r, half = jc // 2, jc % 2
                                for sub in range(2):
                                    h0 = HD * sub
                                    nc.tensor.matmul(
                                        s_pt[sub][:, u, :],
                                        kp[h0 : h0 + HD, r, P * half : P * half + P],
                                        qhT[h0 : h0 + HD, hp, TPB * b : TPB * b + TPB],
                                        start=True, stop=True,
                                    )
                            for f in range(4):
                                nc.tensor.matmul(
                                    dum_pt[0:1, :],
                                    ones_col[:],
                                    expm[:, f, :],
                                    start=True, stop=True,
                                    skip_group_check=True,
                                )
                            for sub in range(2):
                                attnE = aep.tile([P, 4, TPB], BF16, tag="ae")
                                nc.scalar.activation(
                                    out=attnE[:], in_=s_pt[sub][:], func=Exp
                                )
                                attnT = atp.tile([P, 4, TPB], BF16, tag="at")
                                nc.vector.tensor_tensor(
                                    attnT[:],
                                    attnE[:],
                                    expm[:, 4 * g : 4 * g + 4, TPB * b : TPB * b + TPB],
                                    MUL,
                                )
                                for u in range(4):
                                    jc = 4 * g + u
                                    nc.tensor.matmul(
                                        av_pt[sub][0 : HD + 1, :],
                                        vh_sb[:, jc // 2, jc % 2, sub, 0 : HD + 1],
                                        attnT[:, u, :],
                                        start=(g == 0 and u == 0),
                                        stop=(g == 3 and u == 3),
                                    )
                        norm_pair(b, hp, av_pt)
                        if b == 0 and hp == 7:
                            outproj(0)
                            done_b0 = True
                    assert done_b0
                    outproj(1)

    nc.finalize()
    return nc


def _bf16(x):
    x = np.ascontiguousarray(np.asarray(x, np.float32))
    u = x.view(np.uint32)
    r = ((u >> 16) & 1).astype(np.uint32)
    return ((u + 0x7FFF + r) & 0xFFFF0000).view(np.float32)


def _host_prep(x, mask, ln_g, ln_b, w_qkv, b_qkv, in_w, in_b, out_w, out_b):
    f32 = np.float32
    import ml_dtypes

    def to_bf(a):
        return np.asarray(a, np.float32).astype(ml_dtypes.bfloat16)

    perm = np.concatenate([np.arange(0, D, 2), np.arange(1, D, 2)])
    W1 = (w_qkv * ln_g[None, :]).astype(f32)
    b1 = (b_qkv + w_qkv @ ln_b).astype(f32)
    W1q, W1k, W1v = W1[0:D], W1[D : 2 * D], W1[2 * D :]
    b1q, b1k, b1v = b1[0:D], b1[D : 2 * D], b1[2 * D :]
    w1qkT = _bf16(np.concatenate([W1q[perm], W1k[perm]], axis=0).T)  # (D,2D)
    c1qk = np.stack(
        [-w1qkT.sum(axis=0), np.concatenate([b1q[perm], b1k[perm]])]
    ).astype(f32)  # (2, 2D): row0=-s1 row1=b1
    w1vT = _bf16(W1v.T)
    c1v = np.stack([-w1vT.sum(axis=0), b1v]).astype(f32)

    wq, wk, wv = in_w[0:D], in_w[D : 2 * D], in_w[2 * D :]
    bq, bk, bv = in_b[0:D], in_b[D : 2 * D], in_b[2 * D :]
    SC = 1.0 / np.sqrt(HD)
    w2q = np.ascontiguousarray((wq * SC).T[perm])
    w2k = np.ascontiguousarray(wk.T[perm])
    w2T = np.concatenate([w2q, w2k], axis=1).astype(f32)
    b2r = np.concatenate([bq * SC, bk]).reshape(1, 2 * D).astype(f32)
    wvT2 = np.ascontiguousarray(wv.T).astype(f32)
    bvr = bv.reshape(1, D).astype(f32)
    owT = np.ascontiguousarray(out_w.T).astype(f32)
    outbr = out_b.reshape(1, D).astype(f32)

    inv_freq = 1.0 / (THETA ** (np.arange(0, D, 2, dtype=np.float64) / D))

    shared = dict(
        w1qkT=to_bf(w1qkT), w1vT=to_bf(w1vT), c1qk=to_bf(c1qk), c1v=to_bf(c1v),
        w2T=to_bf(w2T), b2=to_bf(b2r), wvT=to_bf(wvT2), bvr=to_bf(bvr),
        owT=to_bf(owT), outb=to_bf(outbr),
    )
    in_maps = []
    for c in range(NCORES):
        rows = slice(TPB * c, TPB * c + TPB)
        xc = np.ascontiguousarray(
            np.concatenate([x[0, rows], x[1, rows]], axis=0).T
        )
        mc = np.ascontiguousarray(
            np.concatenate([mask[0, rows].T, mask[1, rows].T], axis=1)
        )
        pos = np.arange(TPB * c, TPB * c + TPB, dtype=np.float64)
        ang = inv_freq[:, None] * pos[None, :]  # (512, 256)
        cosc = np.cos(ang)
        sinc = np.sin(ang)
        m = dict(shared)
        m["xT"] = to_bf(xc)
        m["maskT"] = to_bf(mc)
        m["cosT"] = to_bf(np.concatenate([cosc, cosc], axis=1))
        m["sinT"] = to_bf(np.concatenate([sinc, sinc], axis=1))
        in_maps.append(m)
    return in_maps


def kernel(**inputs):
    if "nc" not in _cached:
        _cached["nc"] = _build_module()
    nc = _cached["nc"]
    in_maps = _host_prep(**inputs)
    res = run_bass_kernel_spmd(nc, in_maps, list(range(NCORES)), trace=TRACE)
    _cached["last_result"] = res
    out = np.empty((B, S, D), dtype=np.float32)
    for c in range(NCORES):
        o = res.results[c]["outT"]  # (D, 512)
        rows = slice(TPB * c, TPB * c + TPB)
        out[0, rows] = o[:, 0:TPB].T
        out[1, rows] = o[:, TPB : 2 * TPB].T
    return out
